# revision 20
# baseline (speedup 1.0000x reference)
"""Trainium2 Bass kernel for AttentionAugmentedNSDE.

Model (B=64, T=512, D=H=128, L=256):
  1. single-head scaled-dot-product self-attention over x (B,T,D)
  2. Euler-Maruyama neural SDE over T-1 steps: y' = y + drift(y)*dt + diff(y)*(sqrt(dt)*dw)
     drift/diff: Linear(128->256) -> [Linear(256,256)+ReLU]*2 -> Linear(256->128)+Sigmoid
  3. fusion MLP on concat([context, hidden]) per token.

Strategy:
  - Data-parallel over batch: B=64 -> 8 cores x 8 samples, no collectives.
  - Algebraic folds (host, float64): the first Linear of each SDE net has no
    activation so W_in@W_fc0 folds into one 128->256 layer; same for
    fc_in@fc_block[0] in the fusion MLP; attention 1/sqrt(D) folded into Wq;
    v-bias folded to after-context (softmax rows sum to 1).
  - Feature-major layout on chip (features on partitions, batch/time on free
    dim); weights are the stationary matmul operand.
  - SDE scan in fp16 (weights+activations; fp32 PSUM accumulation + fp32
    state). The scan is latency-bound on the per-step dependency chain, so
    both nets share joint PSUM tiles (one elementwise op per layer), biases
    are preloaded into PSUM off the critical path, and the attention phase is
    interleaved into the scan to fill engine gaps and keep the PE clock warm.
"""

import sys

import numpy as np

_TRN = "/opt/trn_rl_repo"
if _TRN not in sys.path:
    sys.path.insert(0, _TRN)

B, T, D, H, L = 64, 512, 128, 128, 256
NCORES = 8
BPC = B // NCORES          # batches per core
NSTEPS = T - 1             # 511 scan steps
DT = np.float32(1.0 / NSTEPS)
SQDT = np.float32(np.sqrt(DT))
SCAN_DT = "f16"            # "f16" or "f8" — low-precision dtype for the SDE scan

_prog_cache = {}


def _f(a):
    return np.ascontiguousarray(np.asarray(a, dtype=np.float64))


def _rep8(col):
    return np.tile(np.asarray(col, np.float32).reshape(-1, 1), (1, BPC))


def fold_params(params):
    """Host-side weight folding in float64, cast to on-chip dtypes."""
    w = {}
    s = 1.0 / np.sqrt(D)
    attn = params["attn"]
    w["wq"] = (_f(attn["q"]["w"]) * s).astype(np.float32)
    w["bq"] = (_f(attn["q"]["b"]) * s).astype(np.float32).reshape(D, 1)
    w["wk"] = _f(attn["k"]["w"]).astype(np.float32)
    w["bk"] = _f(attn["k"]["b"]).astype(np.float32).reshape(D, 1)
    w["wv"] = _f(attn["v"]["w"]).astype(np.float32)
    w["bv"] = _f(attn["v"]["b"]).astype(np.float32).reshape(D, 1)

    if SCAN_DT == "f8":
        import ml_dtypes
        np16 = ml_dtypes.float8_e4m3
    else:
        np16 = np.float16
    bias = {}
    for tag, net in (("d", params["drift"]), ("f", params["diff"])):
        win, bin_ = _f(net["in"]["w"]), _f(net["in"]["b"])
        w0, b0 = _f(net["fc"][0]["w"]), _f(net["fc"][0]["b"])
        w1, b1 = _f(net["fc"][1]["w"]), _f(net["fc"][1]["b"])
        wo, bo = _f(net["out"]["w"]), _f(net["out"]["b"])
        w[f"w{tag}1"] = (win @ w0).astype(np16)                        # 128x256
        w[f"w{tag}1t"] = (float(DT) * (win @ w0)).astype(np16)         # dt-scaled
        bias[f"{tag}1"] = (bin_ @ w0 + b0).astype(np.float32)
        w[f"w{tag}2"] = w1.astype(np16)                                # 256x256
        bias[f"{tag}2"] = b1.astype(np.float32)
        w[f"w{tag}3"] = wo.astype(np16)                                # 256x128
        bias[f"{tag}3"] = bo.astype(np.float32)
    # PSUM bias preload tiles: joint layout [d-m0 | d-m1 | f-m0 | f-m1] x 8 cols
    w["bw1"] = np.concatenate(
        [_rep8(bias["d1"][:128]), _rep8(bias["d1"][128:]),
         _rep8(bias["f1"][:128]), _rep8(bias["f1"][128:])], axis=1)
    w["bw2"] = np.concatenate(
        [_rep8(bias["d2"][:128]), _rep8(bias["d2"][128:]),
         _rep8(bias["f2"][:128]), _rep8(bias["f2"][128:])], axis=1)
    w["bw3"] = np.concatenate([_rep8(bias["f3"]), _rep8(bias["d3"])], axis=1)

    wfi, bfi = _f(params["fc_in"]["w"]), _f(params["fc_in"]["b"])
    wb0, bb0 = _f(params["fc_block"][0]["w"]), _f(params["fc_block"][0]["b"])
    wb1, bb1 = _f(params["fc_block"][1]["w"]), _f(params["fc_block"][1]["b"])
    wo, bo = _f(params["fc_out"]["w"]), _f(params["fc_out"]["b"])
    w["wu1"] = (wfi @ wb0).astype(np.float32)                          # 256x256
    w["bu1"] = (bfi @ wb0 + bb0).astype(np.float32).reshape(L, 1)
    w["wu2"] = wb1.astype(np.float32)                                  # 256x256
    w["bu2"] = bb1.astype(np.float32).reshape(L, 1)
    w["wu3"] = wo.astype(np.float32)                                   # 256x128
    w["bu3"] = bo.astype(np.float32).reshape(D, 1)
    return w


def build_program(n_steps=NSTEPS, do_scan=True, do_attn=True, debug=False,
                  scan_dt=None, attn_stride=56, FUSE_ILV=False):
    scan_dt = scan_dt or SCAN_DT
    import concourse.bacc as bacc
    import concourse.tile as tile
    from concourse import masks, mybir
    from contextlib import ExitStack

    f32 = mybir.dt.float32
    f16 = {"f16": mybir.dt.float16, "f8": mybir.dt.float8e4}[scan_dt]
    AF = mybir.ActivationFunctionType
    ALU = mybir.AluOpType

    Tloc = n_steps + 1

    nc = bacc.Bacc("TRN2", target_bir_lowering=False, debug=False)

    # ---- DRAM I/O ----
    x_d = nc.dram_tensor("x", (BPC, T, D), f32, kind="ExternalInput")
    nz_d = nc.dram_tensor("noise", (NSTEPS, BPC, H), f32, kind="ExternalInput")
    dram = {}
    for name, shape, dt in [
        ("wq", (D, D), f32), ("bq", (D, 1), f32),
        ("wk", (D, D), f32), ("bk", (D, 1), f32),
        ("wv", (D, D), f32), ("bv", (D, 1), f32),
        ("wd1", (H, L), f16), ("wd2", (L, L), f16), ("wd3", (L, H), f16),
        ("wf1", (H, L), f16), ("wf2", (L, L), f16), ("wf3", (L, H), f16),
        ("wd1t", (H, L), f16), ("wf1t", (H, L), f16),
        ("bw1", (128, 4 * BPC), f32), ("bw2", (128, 4 * BPC), f32),
        ("bw3", (128, 2 * BPC), f32),
        ("wu1", (L, L), f32), ("bu1", (L, 1), f32),
        ("wu2", (L, L), f32), ("bu2", (L, 1), f32),
        ("wu3", (L, D), f32), ("bu3", (D, 1), f32),
    ]:
        dram[name] = nc.dram_tensor(name, shape, dt, kind="ExternalInput")
    out_d = nc.dram_tensor("out", (BPC, T, D), f32, kind="ExternalOutput")
    if debug:
        hidT_d = nc.dram_tensor("dbg_hidT", (H, BPC, Tloc), f32, kind="ExternalOutput")
        ctxT_d = nc.dram_tensor("dbg_ctxT", (BPC, D, T), f32, kind="ExternalOutput")

    with tile.TileContext(nc) as tc, ExitStack() as octx:
        const = octx.enter_context(tc.tile_pool(name="const", bufs=1))

        ident = const.tile([128, 128], f32)
        masks.make_identity(nc, ident[:])

        sb = {}
        for name in ("wq", "wk", "wv"):
            sb[name] = const.tile([D, D], f32, name=name)
            nc.sync.dma_start(out=sb[name], in_=dram[name].ap())
        for name in ("bq", "bk", "bv", "bu3"):
            sb[name] = const.tile([128, 1], f32, name=name)
            nc.sync.dma_start(out=sb[name], in_=dram[name].ap())
        for name in ("bw1", "bw2", "bw3"):
            shape = [128, 4 * BPC] if name != "bw3" else [128, 2 * BPC]
            sb[name] = const.tile(shape, f32, name=name)
            nc.sync.dma_start(out=sb[name], in_=dram[name].ap())
        # SDE f16 weights; K>128 stored [128, kc, M]
        for tag in ("d", "f"):
            sb[f"w{tag}1"] = const.tile([H, L], f16, name=f"w{tag}1")
            nc.sync.dma_start(out=sb[f"w{tag}1"], in_=dram[f"w{tag}1"].ap())
            sb[f"w{tag}1t"] = const.tile([H, L], f16, name=f"w{tag}1t")
            nc.sync.dma_start(out=sb[f"w{tag}1t"], in_=dram[f"w{tag}1t"].ap())
            sb[f"w{tag}2"] = const.tile([128, 2, L], f16, name=f"w{tag}2")
            nc.sync.dma_start(
                out=sb[f"w{tag}2"],
                in_=dram[f"w{tag}2"].ap().rearrange("(c p) m -> p c m", p=128))
            sb[f"w{tag}3"] = const.tile([128, 2, H], f16, name=f"w{tag}3")
            nc.sync.dma_start(
                out=sb[f"w{tag}3"],
                in_=dram[f"w{tag}3"].ap().rearrange("(c p) m -> p c m", p=128))
        # fusion f32 weights
        for name in ("wu1", "wu2"):
            sb[name] = const.tile([128, 2, L], f32, name=name)
            nc.sync.dma_start(
                out=sb[name],
                in_=dram[name].ap().rearrange("(c p) m -> p c m", p=128))
        sb["wu3"] = const.tile([128, 2, D], f32, name="wu3")
        nc.sync.dma_start(
            out=sb["wu3"], in_=dram["wu3"].ap().rearrange("(c p) m -> p c m", p=128))
        for name in ("bu1", "bu2"):
            sb[name] = const.tile([128, 2], f32, name=name)
            nc.sync.dma_start(
                out=sb[name],
                in_=dram[name].ap().rearrange("(c p) one -> p (c one)", p=128))

        # persistent state
        hidden = const.tile([H, BPC, Tloc], f32)     # hidden^T, batch-major cols
        noiseT = const.tile([H, n_steps * BPC], f32)  # sqrt(dt)*dw, feature-major
        ctxTs = const.tile([D, BPC, T], f32)          # context^T per batch

        # pools shared by scan + attention so they can interleave
        scn = octx.enter_context(tc.tile_pool(name="scn", bufs=2))
        scnp = octx.enter_context(tc.tile_pool(name="scnp", bufs=1, space="PSUM"))
        atp = octx.enter_context(tc.tile_pool(name="atp", bufs=2))
        apb = octx.enter_context(tc.tile_pool(name="apb", bufs=1, space="PSUM"))
        aps = octx.enter_context(tc.tile_pool(name="aps", bufs=1, space="PSUM"))
        ntp = octx.enter_context(tc.tile_pool(name="ntp", bufs=3))

        def emit_noise_block(t0):
            tcnt = min(16, n_steps - t0)
            rows = tcnt * BPC
            nz = ntp.tile([128, H], f32, tag="nz")
            nc.sync.dma_start(
                out=nz[:rows, :],
                in_=nz_d.ap()[t0:t0 + tcnt].rearrange("t b h -> (t b) h"))
            ps = aps.tile([128, 128], f32, tag="tps")
            nc.tensor.transpose(ps[:, :rows], nz[:rows, :], ident[:rows, :rows])
            nc.scalar.activation(
                out=noiseT[:, t0 * BPC: t0 * BPC + rows], in_=ps[:, :rows],
                func=AF.Copy, scale=float(SQDT))

        for t0 in range(0, min(48, n_steps), 16):
            emit_noise_block(t0)

        fusion_done = set()

        def emit_attn(b):
            """Attention part A for batch b (independent of the scan)."""
            xt = atp.tile([128, 4, D], f32, tag="xt")
            nc.sync.dma_start(
                out=xt, in_=x_d.ap()[b].rearrange("(c p) d -> p c d", p=128))
            xT = atp.tile([D, T], f32, tag="xT")
            for c in range(4):
                ps = aps.tile([128, 128], f32, tag="tps")
                nc.tensor.transpose(ps, xt[:, c, :], ident)
                nc.scalar.copy(xT[:, c * 128:(c + 1) * 128], ps)
            qT = atp.tile([D, T], f32, tag="qT")
            kT = atp.tile([D, T], f32, tag="kT")
            for wname, bname, dst in (("wq", "bq", qT), ("wk", "bk", kT)):
                ps = apb.tile([D, T], f32, tag="bps")
                nc.tensor.matmul(ps, lhsT=sb[wname], rhs=xT, start=True, stop=True)
                nc.scalar.activation(out=dst, in_=ps, func=AF.Identity,
                                     bias=sb[bname])
            v = atp.tile([128, 4, D], f32, tag="v")
            for c in range(4):
                ps = aps.tile([128, 128], f32, tag="tps")
                nc.tensor.matmul(ps, lhsT=xT[:, c * 128:(c + 1) * 128],
                                 rhs=sb["wv"], start=True, stop=True)
                nc.scalar.copy(v[:, c, :], ps)
            # scores (t-major) -> exp (+fused row-sum) -> normalize
            P = atp.tile([128, 4, T], f32, tag="P")
            sums = atp.tile([128, 4], f32, tag="sums")
            for c in range(4):
                ps = apb.tile([128, T], f32, tag="bps")
                nc.tensor.matmul(ps, lhsT=qT[:, c * 128:(c + 1) * 128],
                                 rhs=kT, start=True, stop=True)
                nc.scalar.activation(out=P[:, c, :], in_=ps, func=AF.Exp,
                                     accum_out=sums[:, c:c + 1])
            rec = atp.tile([128, 4], f32, tag="rec")
            nc.vector.reciprocal(rec, sums)
            for c in range(4):
                nc.vector.tensor_scalar_mul(P[:, c, :], P[:, c, :],
                                            rec[:, c:c + 1])
            # transpose P -> PT (s-major)
            PT = atp.tile([128, 4, T], f32, tag="PT")
            for tc4 in range(4):
                for sc in range(4):
                    ps = aps.tile([128, 128], f32, tag="tps")
                    nc.tensor.transpose(
                        ps, P[:, tc4, sc * 128:(sc + 1) * 128], ident)
                    nc.scalar.copy(PT[:, sc, tc4 * 128:(tc4 + 1) * 128], ps)
            # context^T = sum_s v[s,:]^T P^T[s,:] (+ bv)
            psc = apb.tile([D, T], f32, tag="bps")
            for sc in range(4):
                nc.tensor.matmul(psc, lhsT=v[:, sc, :], rhs=PT[:, sc, :],
                                 start=(sc == 0), stop=(sc == 3))
            nc.scalar.activation(out=ctxTs[:, b, :], in_=psc, func=AF.Identity,
                                 bias=sb["bv"])
            if debug:
                nc.sync.dma_start(out=ctxT_d.ap()[b], in_=ctxTs[:, b, :])

        TH = T // 2

        def emit_fusion(b, half):
            tlo = half * TH
            hu1 = atp.tile([128, 2, TH], f32, tag="hu1")
            for mi in range(2):
                ps = scnp.tile([128, TH], f32, tag="pj1", bufs=2)
                nc.tensor.matmul(
                    ps, lhsT=sb["wu1"][:, 0, mi * 128:(mi + 1) * 128],
                    rhs=ctxTs[:, b, tlo:tlo + TH], start=True, stop=False)
                nc.tensor.matmul(
                    ps, lhsT=sb["wu1"][:, 1, mi * 128:(mi + 1) * 128],
                    rhs=hidden[:, b, tlo:tlo + TH], start=False, stop=True)
                nc.vector.tensor_scalar(
                    out=hu1[:, mi, :], in0=ps, scalar1=sb["bu1"][:, mi:mi + 1],
                    scalar2=0.0, op0=ALU.add, op1=ALU.max)
            hu2 = atp.tile([128, 2, TH], f32, tag="hu2")
            for mi in range(2):
                ps = scnp.tile([128, TH], f32, tag="pj2", bufs=2)
                nc.tensor.matmul(
                    ps, lhsT=sb["wu2"][:, 0, mi * 128:(mi + 1) * 128],
                    rhs=hu1[:, 0, :], start=True, stop=False)
                nc.tensor.matmul(
                    ps, lhsT=sb["wu2"][:, 1, mi * 128:(mi + 1) * 128],
                    rhs=hu1[:, 1, :], start=False, stop=True)
                nc.vector.tensor_scalar(
                    out=hu2[:, mi, :], in0=ps, scalar1=sb["bu2"][:, mi:mi + 1],
                    scalar2=0.0, op0=ALU.add, op1=ALU.max)
            pso = scnp.tile([D, TH], f32, tag="pj3", bufs=2)
            nc.tensor.matmul(pso, lhsT=sb["wu3"][:, 0, :], rhs=hu2[:, 0, :],
                             start=True, stop=False)
            nc.tensor.matmul(pso, lhsT=sb["wu3"][:, 1, :], rhs=hu2[:, 1, :],
                             start=False, stop=True)
            outT = atp.tile([D, TH], f32, tag="outT")
            nc.scalar.activation(out=outT, in_=pso, func=AF.Identity,
                                 bias=sb["bu3"])
            ot = atp.tile([128, 2, D], f32, tag="ot")
            for c in range(2):
                ps = aps.tile([128, 128], f32, tag="tps")
                nc.tensor.transpose(ps, outT[:, c * 128:(c + 1) * 128], ident)
                nc.scalar.copy(ot[:, c, :], ps)
            nc.sync.dma_start(
                out=out_d.ap()[b][tlo:tlo + TH].rearrange("(c p) d -> p c d", p=128),
                in_=ot)

        # ---- phase 2: SDE scan (with attention part A interleaved) ----
        attn_queue = list(range(BPC)) if (do_attn and n_steps == T - 1) else []
        if do_scan:
            nc.vector.memset(hidden[:, :, 0], 0.0)
            y16_0 = scn.tile([H, BPC], f16, tag="y16")
            nc.vector.memset(y16_0, 0.0)
            y16 = y16_0
            W1 = {t: sb[f"w{t}1"] for t in ("d", "f")}
            W2 = {t: sb[f"w{t}2"] for t in ("d", "f")}
            W3 = {t: sb[f"w{t}3"] for t in ("d", "f")}
            # fc1 is linear in the update, so p1(t+1) accumulates
            # bias + W1.y(t) (early) + dt.W1.sigd + W1.(sigf*dw) (late);
            # only the sigf*dw term is on the step's critical path.
            def fc1_accum(dst, rhs, suffix="", stop=False):
                for ni, tag in enumerate(("d", "f")):
                    for mi in range(2):
                        nc.tensor.matmul(
                            dst[:, (ni * 2 + mi) * BPC:(ni * 2 + mi + 1) * BPC],
                            lhsT=sb[f"w{tag}1{suffix}"][:, mi * 128:(mi + 1) * 128],
                            rhs=rhs, start=False, stop=stop,
                            skip_group_check=True)

            p1 = scnp.tile([128, 4 * BPC], f32, tag="pj1", bufs=2)
            nc.vector.tensor_copy(p1, sb["bw1"])
            fc1_accum(p1, y16, stop=True)
            for t in range(n_steps):
                if t % 16 == 0 and t + 48 < n_steps:
                    emit_noise_block(t + 48)
                if attn_queue and t == 24 + (BPC - len(attn_queue)) * attn_stride:
                    emit_attn(attn_queue.pop(0))
                if (FUSE_ILV and do_attn and n_steps == T - 1 and t >= 280
                        and t % 16 == 8 and len(fusion_done) < BPC):
                    b = len(fusion_done)
                    emit_fusion(b, 0)
                    fusion_done.add((b, 0))
                # start next step's p1 with the terms known early
                p1n = scnp.tile([128, 4 * BPC], f32, tag="pj1", bufs=2)
                nc.vector.tensor_copy(p1n, sb["bw1"])
                fc1_accum(p1n, y16)
                # per-net relu halves: each net's fc2 starts after its own half
                h1 = scn.tile([128, 4 * BPC], f16, tag="h1")
                nc.vector.tensor_scalar(out=h1[:, 0:2 * BPC], in0=p1[:, 0:2 * BPC],
                                        scalar1=0.0, scalar2=None, op0=ALU.max)
                nc.vector.tensor_scalar(out=h1[:, 2 * BPC:], in0=p1[:, 2 * BPC:],
                                        scalar1=0.0, scalar2=None, op0=ALU.max)
                p2 = scnp.tile([128, 4 * BPC], f32, tag="pj2", bufs=2)
                nc.scalar.copy(p2, sb["bw2"])
                for ni, tag in enumerate(("d", "f")):
                    for mi in range(2):
                        reg = p2[:, (ni * 2 + mi) * BPC:(ni * 2 + mi + 1) * BPC]
                        nc.tensor.matmul(
                            reg, lhsT=W2[tag][:, 0, mi * 128:(mi + 1) * 128],
                            rhs=h1[:, ni * 2 * BPC:(ni * 2 + 1) * BPC],
                            start=False, stop=False, skip_group_check=True)
                        nc.tensor.matmul(
                            reg, lhsT=W2[tag][:, 1, mi * 128:(mi + 1) * 128],
                            rhs=h1[:, (ni * 2 + 1) * BPC:(ni * 2 + 2) * BPC],
                            start=False, stop=True, skip_group_check=True)
                h2 = scn.tile([128, 4 * BPC], f16, tag="h2")
                nc.vector.tensor_scalar(out=h2[:, 2 * BPC:], in0=p2[:, 2 * BPC:],
                                        scalar1=0.0, scalar2=None, op0=ALU.max)
                nc.vector.tensor_scalar(out=h2[:, 0:2 * BPC], in0=p2[:, 0:2 * BPC],
                                        scalar1=0.0, scalar2=None, op0=ALU.max)
                # p3 joint [f | d]: diff first so the noise-multiply
                # (which gates y16) starts as early as possible
                p3 = scnp.tile([H, 2 * BPC], f32, tag="pj3", bufs=2)
                nc.scalar.copy(p3, sb["bw3"])
                hslc = {"d": (h2[:, 0:BPC], h2[:, BPC:2 * BPC]),
                        "f": (h2[:, 2 * BPC:3 * BPC], h2[:, 3 * BPC:4 * BPC])}
                for ni, tag in enumerate(("f", "d")):
                    reg = p3[:, ni * BPC:(ni + 1) * BPC]
                    nc.tensor.matmul(
                        reg, lhsT=W3[tag][:, 0, :], rhs=hslc[tag][0],
                        start=False, stop=False, skip_group_check=True)
                    nc.tensor.matmul(
                        reg, lhsT=W3[tag][:, 1, :], rhs=hslc[tag][1],
                        start=False, stop=True, skip_group_check=True)
                sg = scn.tile([H, 2 * BPC], f32, tag="sg")
                nc.scalar.activation(out=sg, in_=p3, func=AF.Sigmoid)
                # critical path: fp16 delta terms feed next step's fc1 PSUM
                ftmp16 = scn.tile([H, BPC], f16, tag="ftmp16")
                nc.vector.tensor_mul(
                    ftmp16, sg[:, 0:BPC], noiseT[:, t * BPC:(t + 1) * BPC])
                sgd16 = scn.tile([H, BPC], f16, tag="sgd16")
                nc.vector.tensor_copy(sgd16, sg[:, BPC:2 * BPC])
                for ni, tag in enumerate(("d", "f")):
                    for mi in range(2):
                        sl = p1n[:, (ni * 2 + mi) * BPC:(ni * 2 + mi + 1) * BPC]
                        nc.tensor.matmul(
                            sl, lhsT=sb[f"w{tag}1t"][:, mi * 128:(mi + 1) * 128],
                            rhs=sgd16, start=False, stop=False,
                            skip_group_check=True)
                        nc.tensor.matmul(
                            sl, lhsT=sb[f"w{tag}1"][:, mi * 128:(mi + 1) * 128],
                            rhs=ftmp16, start=False, stop=True,
                            skip_group_check=True)
                # off critical path: exact fp32 state update
                ftmp = scn.tile([H, BPC], f32, tag="ftmp")
                nc.gpsimd.tensor_mul(
                    ftmp, sg[:, 0:BPC], noiseT[:, t * BPC:(t + 1) * BPC])
                ytmp = scn.tile([H, BPC], f32, tag="ytmp")
                nc.vector.scalar_tensor_tensor(
                    out=ytmp, in0=sg[:, BPC:2 * BPC], scalar=float(DT),
                    in1=hidden[:, :, t], op0=ALU.mult, op1=ALU.add)
                nc.gpsimd.tensor_add(hidden[:, :, t + 1], ytmp, ftmp)
                y16 = scn.tile([H, BPC], f16, tag="y16")
                nc.vector.tensor_add(y16, ytmp, ftmp)
                p1 = p1n
        else:
            nc.vector.memset(hidden[:, :, :], 0.0)
        for b in attn_queue:
            emit_attn(b)
        if debug:
            nc.sync.dma_start(out=hidT_d.ap(), in_=hidden[:, :, :])

        # ---- phase 3 epilogue: remaining fusion halves ----
        if do_attn:
            assert Tloc == T
            for b in range(BPC):
                if (b, 0) not in fusion_done:
                    emit_fusion(b, 0)
                emit_fusion(b, 1)
    nc.compile()
    return nc


def _get_program():
    key = ("full", SCAN_DT)
    if key not in _prog_cache:
        _prog_cache[key] = build_program()
    return _prog_cache[key]


def kernel(x, noise, params):
    from concourse import bass_utils

    x = np.ascontiguousarray(np.asarray(x, dtype=np.float32))
    noise = np.ascontiguousarray(np.asarray(noise, dtype=np.float32))
    w = fold_params(params)

    nc = _get_program()
    in_maps = []
    for c in range(NCORES):
        m = dict(w)
        m["x"] = np.ascontiguousarray(x[c * BPC:(c + 1) * BPC])
        m["noise"] = np.ascontiguousarray(noise[:, c * BPC:(c + 1) * BPC])
        in_maps.append(m)
    res = bass_utils.run_bass_kernel_spmd(nc, in_maps, core_ids=list(range(NCORES)))
    return np.concatenate([res.results[c]["out"] for c in range(NCORES)], axis=0)


if __name__ == "__main__":
    import pickle, time
    x = np.load("/tmp/x.npy")
    noise = np.load("/tmp/noise.npy")
    with open("/tmp/params.pkl", "rb") as f:
        params = pickle.load(f)
    t0 = time.time()
    out = kernel(x, noise, params)
    print("kernel() wall:", time.time() - t0)
    ref = np.load("/tmp/ref_np.npy")
    err = np.abs(out - ref)
    print("absmax:", err.max(), "rel:", err.max() / np.abs(ref).max())


# revision 23
# speedup vs baseline: 1.0407x; 1.0407x over previous
"""Trainium2 Bass kernel for AttentionAugmentedNSDE.

Model (B=64, T=512, D=H=128, L=256):
  1. single-head scaled-dot-product self-attention over x (B,T,D)
  2. Euler-Maruyama neural SDE over T-1 steps: y' = y + drift(y)*dt + diff(y)*(sqrt(dt)*dw)
     drift/diff: Linear(128->256) -> [Linear(256,256)+ReLU]*2 -> Linear(256->128)+Sigmoid
  3. fusion MLP on concat([context, hidden]) per token.

Strategy:
  - Data-parallel over batch: B=64 -> 8 cores x 8 samples, no collectives.
  - Algebraic folds (host, float64): the first Linear of each SDE net has no
    activation so W_in@W_fc0 folds into one 128->256 layer; same for
    fc_in@fc_block[0] in the fusion MLP; attention 1/sqrt(D) folded into Wq;
    v-bias folded to after-context (softmax rows sum to 1).
  - Feature-major layout on chip (features on partitions, batch/time on free
    dim); weights are the stationary matmul operand.
  - SDE scan in fp16 (weights+activations; fp32 PSUM accumulation + fp32
    state). The scan is latency-bound on the per-step dependency chain, so
    both nets share joint PSUM tiles (one elementwise op per layer), biases
    are preloaded into PSUM off the critical path, and the attention phase is
    interleaved into the scan to fill engine gaps and keep the PE clock warm.
"""

import sys

import numpy as np

_TRN = "/opt/trn_rl_repo"
if _TRN not in sys.path:
    sys.path.insert(0, _TRN)

B, T, D, H, L = 64, 512, 128, 128, 256
NCORES = 8
BPC = B // NCORES          # batches per core
NSTEPS = T - 1             # 511 scan steps
DT = np.float32(1.0 / NSTEPS)
SQDT = np.float32(np.sqrt(DT))
SCAN_DT = "f16"            # "f16" or "f8" — low-precision dtype for the SDE scan

_prog_cache = {}


def _f(a):
    return np.ascontiguousarray(np.asarray(a, dtype=np.float64))


def _rep8(col):
    return np.tile(np.asarray(col, np.float32).reshape(-1, 1), (1, BPC))


def fold_params(params):
    """Host-side weight folding in float64, cast to on-chip dtypes."""
    w = {}
    s = 1.0 / np.sqrt(D)
    attn = params["attn"]
    w["wq"] = (_f(attn["q"]["w"]) * s).astype(np.float32)
    w["bq"] = (_f(attn["q"]["b"]) * s).astype(np.float32).reshape(D, 1)
    w["wk"] = _f(attn["k"]["w"]).astype(np.float32)
    w["bk"] = _f(attn["k"]["b"]).astype(np.float32).reshape(D, 1)
    w["wv"] = _f(attn["v"]["w"]).astype(np.float32)
    w["bv"] = _f(attn["v"]["b"]).astype(np.float32).reshape(D, 1)

    if SCAN_DT == "f8":
        import ml_dtypes
        np16 = ml_dtypes.float8_e4m3
    else:
        np16 = np.float16
    bias = {}
    for tag, net in (("d", params["drift"]), ("f", params["diff"])):
        win, bin_ = _f(net["in"]["w"]), _f(net["in"]["b"])
        w0, b0 = _f(net["fc"][0]["w"]), _f(net["fc"][0]["b"])
        w1, b1 = _f(net["fc"][1]["w"]), _f(net["fc"][1]["b"])
        wo, bo = _f(net["out"]["w"]), _f(net["out"]["b"])
        w[f"w{tag}1"] = (win @ w0).astype(np16)                        # 128x256
        w[f"w{tag}1t"] = (float(DT) * (win @ w0)).astype(np16)         # dt-scaled
        bias[f"{tag}1"] = (bin_ @ w0 + b0).astype(np.float32)
        w[f"w{tag}2"] = w1.astype(np16)                                # 256x256
        bias[f"{tag}2"] = b1.astype(np.float32)
        w[f"w{tag}3"] = wo.astype(np16)                                # 256x128
        bias[f"{tag}3"] = bo.astype(np.float32)
    # PSUM bias preload tiles: joint layout [d-m0 | d-m1 | f-m0 | f-m1] x 8 cols
    w["bw1"] = np.concatenate(
        [_rep8(bias["d1"][:128]), _rep8(bias["d1"][128:]),
         _rep8(bias["f1"][:128]), _rep8(bias["f1"][128:])], axis=1)
    w["bw2"] = np.concatenate(
        [_rep8(bias["d2"][:128]), _rep8(bias["d2"][128:]),
         _rep8(bias["f2"][:128]), _rep8(bias["f2"][128:])], axis=1)
    w["bw3"] = np.concatenate([_rep8(bias["f3"]), _rep8(bias["d3"])], axis=1)

    wfi, bfi = _f(params["fc_in"]["w"]), _f(params["fc_in"]["b"])
    wb0, bb0 = _f(params["fc_block"][0]["w"]), _f(params["fc_block"][0]["b"])
    wb1, bb1 = _f(params["fc_block"][1]["w"]), _f(params["fc_block"][1]["b"])
    wo, bo = _f(params["fc_out"]["w"]), _f(params["fc_out"]["b"])
    w["wu1"] = (wfi @ wb0).astype(np.float32)                          # 256x256
    w["bu1"] = (bfi @ wb0 + bb0).astype(np.float32).reshape(L, 1)
    w["wu2"] = wb1.astype(np.float32)                                  # 256x256
    w["bu2"] = bb1.astype(np.float32).reshape(L, 1)
    w["wu3"] = wo.astype(np.float32)                                   # 256x128
    w["bu3"] = bo.astype(np.float32).reshape(D, 1)
    return w


def build_program(n_steps=NSTEPS, do_scan=True, do_attn=True, debug=False,
                  scan_dt=None, attn_stride=40, FUSE_ILV=True):
    scan_dt = scan_dt or SCAN_DT
    import concourse.bacc as bacc
    import concourse.tile as tile
    from concourse import masks, mybir
    from contextlib import ExitStack

    f32 = mybir.dt.float32
    f16 = {"f16": mybir.dt.float16, "f8": mybir.dt.float8e4}[scan_dt]
    AF = mybir.ActivationFunctionType
    ALU = mybir.AluOpType

    Tloc = n_steps + 1

    nc = bacc.Bacc("TRN2", target_bir_lowering=False, debug=False)

    # ---- DRAM I/O ----
    x_d = nc.dram_tensor("x", (BPC, T, D), f32, kind="ExternalInput")
    nz_d = nc.dram_tensor("noise", (NSTEPS, BPC, H), f32, kind="ExternalInput")
    dram = {}
    for name, shape, dt in [
        ("wq", (D, D), f32), ("bq", (D, 1), f32),
        ("wk", (D, D), f32), ("bk", (D, 1), f32),
        ("wv", (D, D), f32), ("bv", (D, 1), f32),
        ("wd1", (H, L), f16), ("wd2", (L, L), f16), ("wd3", (L, H), f16),
        ("wf1", (H, L), f16), ("wf2", (L, L), f16), ("wf3", (L, H), f16),
        ("wd1t", (H, L), f16), ("wf1t", (H, L), f16),
        ("bw1", (128, 4 * BPC), f32), ("bw2", (128, 4 * BPC), f32),
        ("bw3", (128, 2 * BPC), f32),
        ("wu1", (L, L), f32), ("bu1", (L, 1), f32),
        ("wu2", (L, L), f32), ("bu2", (L, 1), f32),
        ("wu3", (L, D), f32), ("bu3", (D, 1), f32),
    ]:
        dram[name] = nc.dram_tensor(name, shape, dt, kind="ExternalInput")
    out_d = nc.dram_tensor("out", (BPC, T, D), f32, kind="ExternalOutput")
    if debug:
        hidT_d = nc.dram_tensor("dbg_hidT", (H, BPC, Tloc), f32, kind="ExternalOutput")
        ctxT_d = nc.dram_tensor("dbg_ctxT", (BPC, D, T), f32, kind="ExternalOutput")

    with tile.TileContext(nc) as tc, ExitStack() as octx:
        const = octx.enter_context(tc.tile_pool(name="const", bufs=1))

        ident = const.tile([128, 128], f32)
        masks.make_identity(nc, ident[:])

        sb = {}
        for name in ("wq", "wk", "wv"):
            sb[name] = const.tile([D, D], f32, name=name)
            nc.sync.dma_start(out=sb[name], in_=dram[name].ap())
        for name in ("bq", "bk", "bv", "bu3"):
            sb[name] = const.tile([128, 1], f32, name=name)
            nc.sync.dma_start(out=sb[name], in_=dram[name].ap())
        for name in ("bw1", "bw2", "bw3"):
            shape = [128, 4 * BPC] if name != "bw3" else [128, 2 * BPC]
            sb[name] = const.tile(shape, f32, name=name)
            nc.sync.dma_start(out=sb[name], in_=dram[name].ap())
        # SDE f16 weights; K>128 stored [128, kc, M]
        for tag in ("d", "f"):
            sb[f"w{tag}1"] = const.tile([H, L], f16, name=f"w{tag}1")
            nc.sync.dma_start(out=sb[f"w{tag}1"], in_=dram[f"w{tag}1"].ap())
            sb[f"w{tag}1t"] = const.tile([H, L], f16, name=f"w{tag}1t")
            nc.sync.dma_start(out=sb[f"w{tag}1t"], in_=dram[f"w{tag}1t"].ap())
            sb[f"w{tag}2"] = const.tile([128, 2, L], f16, name=f"w{tag}2")
            nc.sync.dma_start(
                out=sb[f"w{tag}2"],
                in_=dram[f"w{tag}2"].ap().rearrange("(c p) m -> p c m", p=128))
            sb[f"w{tag}3"] = const.tile([128, 2, H], f16, name=f"w{tag}3")
            nc.sync.dma_start(
                out=sb[f"w{tag}3"],
                in_=dram[f"w{tag}3"].ap().rearrange("(c p) m -> p c m", p=128))
        # fusion f32 weights
        for name in ("wu1", "wu2"):
            sb[name] = const.tile([128, 2, L], f32, name=name)
            nc.sync.dma_start(
                out=sb[name],
                in_=dram[name].ap().rearrange("(c p) m -> p c m", p=128))
        sb["wu3"] = const.tile([128, 2, D], f32, name="wu3")
        nc.sync.dma_start(
            out=sb["wu3"], in_=dram["wu3"].ap().rearrange("(c p) m -> p c m", p=128))
        for name in ("bu1", "bu2"):
            sb[name] = const.tile([128, 2], f32, name=name)
            nc.sync.dma_start(
                out=sb[name],
                in_=dram[name].ap().rearrange("(c p) one -> p (c one)", p=128))

        # persistent state; hidden split at t=SPLIT so the fusion's first
        # T-half has a clean whole-tile dependency (no mid-scan subtile races)
        SPLIT = T // 2 if n_steps == T - 1 else Tloc
        hid_a = const.tile([H, BPC, SPLIT], f32)
        hid_b = const.tile([H, BPC, max(Tloc - SPLIT, 1)], f32)

        def hid(c):
            return (hid_a[:, :, c] if c < SPLIT else hid_b[:, :, c - SPLIT])

        def hid_batch(b, half):
            return (hid_a if half == 0 else hid_b)[:, b, :]
        noiseT = const.tile([H, n_steps * BPC], f32)  # sqrt(dt)*dw, feature-major
        ctxT_l = [const.tile([D, T], f32, name=f"ctxT{b}") for b in range(BPC)]

        # pools shared by scan + attention so they can interleave
        scn = octx.enter_context(tc.tile_pool(name="scn", bufs=2))
        scnp = octx.enter_context(tc.tile_pool(name="scnp", bufs=1, space="PSUM"))
        atp = octx.enter_context(tc.tile_pool(name="atp", bufs=2))
        apb = octx.enter_context(tc.tile_pool(name="apb", bufs=1, space="PSUM"))
        aps = octx.enter_context(tc.tile_pool(name="aps", bufs=1, space="PSUM"))
        ntp = octx.enter_context(tc.tile_pool(name="ntp", bufs=3))

        def emit_noise_block(t0):
            tcnt = min(16, n_steps - t0)
            rows = tcnt * BPC
            nz = ntp.tile([128, H], f32, tag="nz")
            nc.sync.dma_start(
                out=nz[:rows, :],
                in_=nz_d.ap()[t0:t0 + tcnt].rearrange("t b h -> (t b) h"))
            ps = aps.tile([128, 128], f32, tag="tps")
            nc.tensor.transpose(ps[:, :rows], nz[:rows, :], ident[:rows, :rows])
            nc.scalar.activation(
                out=noiseT[:, t0 * BPC: t0 * BPC + rows], in_=ps[:, :rows],
                func=AF.Copy, scale=float(SQDT))

        for t0 in range(0, min(48, n_steps), 16):
            emit_noise_block(t0)

        fusion_done = set()

        def emit_attn(b):
            """Attention part A for batch b (independent of the scan)."""
            xt = atp.tile([128, 4, D], f32, tag="xt")
            nc.sync.dma_start(
                out=xt, in_=x_d.ap()[b].rearrange("(c p) d -> p c d", p=128))
            xT = atp.tile([D, T], f32, tag="xT")
            for c in range(4):
                ps = aps.tile([128, 128], f32, tag="tps")
                nc.tensor.transpose(ps, xt[:, c, :], ident)
                nc.scalar.copy(xT[:, c * 128:(c + 1) * 128], ps)
            qT = atp.tile([D, T], f32, tag="qT")
            kT = atp.tile([D, T], f32, tag="kT")
            for wname, bname, dst in (("wq", "bq", qT), ("wk", "bk", kT)):
                ps = apb.tile([D, T], f32, tag="bps")
                nc.tensor.matmul(ps, lhsT=sb[wname], rhs=xT, start=True, stop=True)
                nc.scalar.activation(out=dst, in_=ps, func=AF.Identity,
                                     bias=sb[bname])
            v = atp.tile([128, 4, D], f32, tag="v")
            for c in range(4):
                ps = aps.tile([128, 128], f32, tag="tps")
                nc.tensor.matmul(ps, lhsT=xT[:, c * 128:(c + 1) * 128],
                                 rhs=sb["wv"], start=True, stop=True)
                nc.scalar.copy(v[:, c, :], ps)
            # scores (t-major) -> exp (+fused row-sum) -> normalize
            P = atp.tile([128, 4, T], f32, tag="P")
            sums = atp.tile([128, 4], f32, tag="sums")
            for c in range(4):
                ps = apb.tile([128, T], f32, tag="bps")
                nc.tensor.matmul(ps, lhsT=qT[:, c * 128:(c + 1) * 128],
                                 rhs=kT, start=True, stop=True)
                nc.scalar.activation(out=P[:, c, :], in_=ps, func=AF.Exp,
                                     accum_out=sums[:, c:c + 1])
            rec = atp.tile([128, 4], f32, tag="rec")
            nc.vector.reciprocal(rec, sums)
            for c in range(4):
                nc.vector.tensor_scalar_mul(P[:, c, :], P[:, c, :],
                                            rec[:, c:c + 1])
            # transpose P -> PT (s-major)
            PT = atp.tile([128, 4, T], f32, tag="PT")
            for tc4 in range(4):
                for sc in range(4):
                    ps = aps.tile([128, 128], f32, tag="tps")
                    nc.tensor.transpose(
                        ps, P[:, tc4, sc * 128:(sc + 1) * 128], ident)
                    nc.scalar.copy(PT[:, sc, tc4 * 128:(tc4 + 1) * 128], ps)
            # context^T = sum_s v[s,:]^T P^T[s,:] (+ bv)
            psc = apb.tile([D, T], f32, tag="bps")
            for sc in range(4):
                nc.tensor.matmul(psc, lhsT=v[:, sc, :], rhs=PT[:, sc, :],
                                 start=(sc == 0), stop=(sc == 3))
            nc.scalar.activation(out=ctxT_l[b], in_=psc, func=AF.Identity,
                                 bias=sb["bv"])
            if debug:
                nc.sync.dma_start(out=ctxT_d.ap()[b], in_=ctxT_l[b])

        TH = T // 2

        def emit_fusion(b, half):
            tlo = half * TH
            hu1 = atp.tile([128, 2, TH], f32, tag="hu1")
            for mi in range(2):
                ps = scnp.tile([128, TH], f32, tag="fps", bufs=2)
                nc.tensor.matmul(
                    ps, lhsT=sb["wu1"][:, 0, mi * 128:(mi + 1) * 128],
                    rhs=ctxT_l[b][:, tlo:tlo + TH], start=True, stop=False)
                nc.tensor.matmul(
                    ps, lhsT=sb["wu1"][:, 1, mi * 128:(mi + 1) * 128],
                    rhs=hid_batch(b, half), start=False, stop=True)
                nc.vector.tensor_scalar(
                    out=hu1[:, mi, :], in0=ps, scalar1=sb["bu1"][:, mi:mi + 1],
                    scalar2=0.0, op0=ALU.add, op1=ALU.max)
            hu2 = atp.tile([128, 2, TH], f32, tag="hu2")
            for mi in range(2):
                ps = scnp.tile([128, TH], f32, tag="fps", bufs=2)
                nc.tensor.matmul(
                    ps, lhsT=sb["wu2"][:, 0, mi * 128:(mi + 1) * 128],
                    rhs=hu1[:, 0, :], start=True, stop=False)
                nc.tensor.matmul(
                    ps, lhsT=sb["wu2"][:, 1, mi * 128:(mi + 1) * 128],
                    rhs=hu1[:, 1, :], start=False, stop=True)
                nc.vector.tensor_scalar(
                    out=hu2[:, mi, :], in0=ps, scalar1=sb["bu2"][:, mi:mi + 1],
                    scalar2=0.0, op0=ALU.add, op1=ALU.max)
            pso = scnp.tile([D, TH], f32, tag="fps", bufs=2)
            nc.tensor.matmul(pso, lhsT=sb["wu3"][:, 0, :], rhs=hu2[:, 0, :],
                             start=True, stop=False)
            nc.tensor.matmul(pso, lhsT=sb["wu3"][:, 1, :], rhs=hu2[:, 1, :],
                             start=False, stop=True)
            outT = atp.tile([D, TH], f32, tag="outT")
            nc.scalar.activation(out=outT, in_=pso, func=AF.Identity,
                                 bias=sb["bu3"])
            ot = atp.tile([128, 2, D], f32, tag="ot")
            for c in range(2):
                ps = aps.tile([128, 128], f32, tag="tps")
                nc.tensor.transpose(ps, outT[:, c * 128:(c + 1) * 128], ident)
                nc.scalar.copy(ot[:, c, :], ps)
            nc.sync.dma_start(
                out=out_d.ap()[b][tlo:tlo + TH].rearrange("(c p) d -> p c d", p=128),
                in_=ot)

        # ---- phase 2: SDE scan (with attention part A interleaved) ----
        attn_queue = list(range(BPC)) if (do_attn and n_steps == T - 1) else []
        if do_scan:
            nc.vector.memset(hid_a[:, :, 0], 0.0)
            y16_0 = scn.tile([H, BPC], f16, tag="y16")
            nc.vector.memset(y16_0, 0.0)
            y16 = y16_0
            W1 = {t: sb[f"w{t}1"] for t in ("d", "f")}
            W2 = {t: sb[f"w{t}2"] for t in ("d", "f")}
            W3 = {t: sb[f"w{t}3"] for t in ("d", "f")}
            # fc1 is linear in the update, so p1(t+1) accumulates
            # bias + W1.y(t) (early) + dt.W1.sigd + W1.(sigf*dw) (late);
            # only the sigf*dw term is on the step's critical path.
            def fc1_accum(dst, rhs, suffix="", stop=False):
                for ni, tag in enumerate(("d", "f")):
                    for mi in range(2):
                        nc.tensor.matmul(
                            dst[:, (ni * 2 + mi) * BPC:(ni * 2 + mi + 1) * BPC],
                            lhsT=sb[f"w{tag}1{suffix}"][:, mi * 128:(mi + 1) * 128],
                            rhs=rhs, start=False, stop=stop,
                            skip_group_check=True)

            p1 = scnp.tile([128, 4 * BPC], f32, tag="pj1", bufs=2)
            nc.vector.tensor_copy(p1, sb["bw1"])
            fc1_accum(p1, y16, stop=True)
            for t in range(n_steps):
                if t % 16 == 0 and t + 48 < n_steps:
                    emit_noise_block(t + 48)
                if attn_queue and t == 24 + (BPC - len(attn_queue)) * attn_stride:
                    emit_attn(attn_queue.pop(0))
                if (FUSE_ILV and do_attn and n_steps == T - 1 and t >= 328
                        and t % 16 == 8 and len(fusion_done) < BPC):
                    b = len(fusion_done)
                    emit_fusion(b, 0)
                    fusion_done.add((b, 0))
                # start next step's p1 with the terms known early
                p1n = scnp.tile([128, 4 * BPC], f32, tag="pj1", bufs=2)
                nc.vector.tensor_copy(p1n, sb["bw1"])
                fc1_accum(p1n, y16)
                h1 = scn.tile([128, 4 * BPC], f16, tag="h1")
                nc.vector.tensor_scalar(out=h1, in0=p1, scalar1=0.0, scalar2=None,
                                        op0=ALU.max)
                p2 = scnp.tile([128, 4 * BPC], f32, tag="pj2", bufs=2)
                nc.scalar.copy(p2, sb["bw2"])
                for ni, tag in enumerate(("d", "f")):
                    for mi in range(2):
                        reg = p2[:, (ni * 2 + mi) * BPC:(ni * 2 + mi + 1) * BPC]
                        nc.tensor.matmul(
                            reg, lhsT=W2[tag][:, 0, mi * 128:(mi + 1) * 128],
                            rhs=h1[:, ni * 2 * BPC:(ni * 2 + 1) * BPC],
                            start=False, stop=False, skip_group_check=True)
                        nc.tensor.matmul(
                            reg, lhsT=W2[tag][:, 1, mi * 128:(mi + 1) * 128],
                            rhs=h1[:, (ni * 2 + 1) * BPC:(ni * 2 + 2) * BPC],
                            start=False, stop=True, skip_group_check=True)
                h2 = scn.tile([128, 4 * BPC], f16, tag="h2")
                nc.vector.tensor_scalar(out=h2, in0=p2, scalar1=0.0, scalar2=None,
                                        op0=ALU.max)
                # p3 joint [f | d]: diff first so the noise-multiply
                # (which gates y16) starts as early as possible
                p3 = scnp.tile([H, 2 * BPC], f32, tag="pj2", bufs=2)
                nc.scalar.copy(p3, sb["bw3"])
                hslc = {"d": (h2[:, 0:BPC], h2[:, BPC:2 * BPC]),
                        "f": (h2[:, 2 * BPC:3 * BPC], h2[:, 3 * BPC:4 * BPC])}
                for ni, tag in enumerate(("f", "d")):
                    reg = p3[:, ni * BPC:(ni + 1) * BPC]
                    nc.tensor.matmul(
                        reg, lhsT=W3[tag][:, 0, :], rhs=hslc[tag][0],
                        start=False, stop=False, skip_group_check=True)
                    nc.tensor.matmul(
                        reg, lhsT=W3[tag][:, 1, :], rhs=hslc[tag][1],
                        start=False, stop=True, skip_group_check=True)
                sg = scn.tile([H, 2 * BPC], f32, tag="sg")
                nc.scalar.activation(out=sg, in_=p3, func=AF.Sigmoid)
                # critical path: fp16 delta terms feed next step's fc1 PSUM
                ftmp16 = scn.tile([H, BPC], f16, tag="ftmp16")
                nc.vector.tensor_mul(
                    ftmp16, sg[:, 0:BPC], noiseT[:, t * BPC:(t + 1) * BPC])
                sgd16 = scn.tile([H, BPC], f16, tag="sgd16")
                nc.vector.tensor_copy(sgd16, sg[:, BPC:2 * BPC])
                for ni, tag in enumerate(("d", "f")):
                    for mi in range(2):
                        sl = p1n[:, (ni * 2 + mi) * BPC:(ni * 2 + mi + 1) * BPC]
                        nc.tensor.matmul(
                            sl, lhsT=sb[f"w{tag}1t"][:, mi * 128:(mi + 1) * 128],
                            rhs=sgd16, start=False, stop=False,
                            skip_group_check=True)
                        nc.tensor.matmul(
                            sl, lhsT=sb[f"w{tag}1"][:, mi * 128:(mi + 1) * 128],
                            rhs=ftmp16, start=False, stop=True,
                            skip_group_check=True)
                # off critical path: exact fp32 state update
                ftmp = scn.tile([H, BPC], f32, tag="ftmp")
                nc.gpsimd.tensor_mul(
                    ftmp, sg[:, 0:BPC], noiseT[:, t * BPC:(t + 1) * BPC])
                ytmp = scn.tile([H, BPC], f32, tag="ytmp")
                nc.vector.scalar_tensor_tensor(
                    out=ytmp, in0=sg[:, BPC:2 * BPC], scalar=float(DT),
                    in1=hid(t), op0=ALU.mult, op1=ALU.add)
                nc.gpsimd.tensor_add(hid(t + 1), ytmp, ftmp)
                y16 = scn.tile([H, BPC], f16, tag="y16")
                nc.vector.tensor_add(y16, ytmp, ftmp)
                p1 = p1n
        else:
            nc.vector.memset(hid_a[:, :, :], 0.0)
            nc.vector.memset(hid_b[:, :, :], 0.0)
        for b in attn_queue:
            emit_attn(b)
        if debug:
            nc.sync.dma_start(out=hidT_d.ap()[:, :, 0:SPLIT], in_=hid_a)
            if Tloc > SPLIT:
                nc.sync.dma_start(out=hidT_d.ap()[:, :, SPLIT:], in_=hid_b)

        # ---- phase 3 epilogue: remaining fusion halves ----
        if do_attn:
            assert Tloc == T
            for b in range(BPC):
                if (b, 0) not in fusion_done:
                    emit_fusion(b, 0)
                emit_fusion(b, 1)
    nc.compile()
    return nc


def _get_program():
    key = ("full", SCAN_DT)
    if key not in _prog_cache:
        _prog_cache[key] = build_program()
    return _prog_cache[key]


def kernel(x, noise, params):
    from concourse import bass_utils

    x = np.ascontiguousarray(np.asarray(x, dtype=np.float32))
    noise = np.ascontiguousarray(np.asarray(noise, dtype=np.float32))
    w = fold_params(params)

    nc = _get_program()
    in_maps = []
    for c in range(NCORES):
        m = dict(w)
        m["x"] = np.ascontiguousarray(x[c * BPC:(c + 1) * BPC])
        m["noise"] = np.ascontiguousarray(noise[:, c * BPC:(c + 1) * BPC])
        in_maps.append(m)
    res = bass_utils.run_bass_kernel_spmd(nc, in_maps, core_ids=list(range(NCORES)))
    return np.concatenate([res.results[c]["out"] for c in range(NCORES)], axis=0)


if __name__ == "__main__":
    import pickle, time
    x = np.load("/tmp/x.npy")
    noise = np.load("/tmp/noise.npy")
    with open("/tmp/params.pkl", "rb") as f:
        params = pickle.load(f)
    t0 = time.time()
    out = kernel(x, noise, params)
    print("kernel() wall:", time.time() - t0)
    ref = np.load("/tmp/ref_np.npy")
    err = np.abs(out - ref)
    print("absmax:", err.max(), "rel:", err.max() / np.abs(ref).max())


# revision 24
# speedup vs baseline: 1.0419x; 1.0012x over previous
"""Trainium2 Bass kernel for AttentionAugmentedNSDE.

Model (B=64, T=512, D=H=128, L=256):
  1. single-head scaled-dot-product self-attention over x (B,T,D)
  2. Euler-Maruyama neural SDE over T-1 steps: y' = y + drift(y)*dt + diff(y)*(sqrt(dt)*dw)
     drift/diff: Linear(128->256) -> [Linear(256,256)+ReLU]*2 -> Linear(256->128)+Sigmoid
  3. fusion MLP on concat([context, hidden]) per token.

Strategy:
  - Data-parallel over batch: B=64 -> 8 cores x 8 samples, no collectives.
  - Algebraic folds (host, float64): the first Linear of each SDE net has no
    activation so W_in@W_fc0 folds into one 128->256 layer; same for
    fc_in@fc_block[0] in the fusion MLP; attention 1/sqrt(D) folded into Wq;
    v-bias folded to after-context (softmax rows sum to 1).
  - Feature-major layout on chip (features on partitions, batch/time on free
    dim); weights are the stationary matmul operand.
  - SDE scan in fp16 (weights+activations; fp32 PSUM accumulation + fp32
    state). The scan is latency-bound on the per-step dependency chain, so
    both nets share joint PSUM tiles (one elementwise op per layer), biases
    are preloaded into PSUM off the critical path, and the attention phase is
    interleaved into the scan to fill engine gaps and keep the PE clock warm.
"""

import sys

import numpy as np

_TRN = "/opt/trn_rl_repo"
if _TRN not in sys.path:
    sys.path.insert(0, _TRN)

B, T, D, H, L = 64, 512, 128, 128, 256
NCORES = 8
BPC = B // NCORES          # batches per core
NSTEPS = T - 1             # 511 scan steps
DT = np.float32(1.0 / NSTEPS)
SQDT = np.float32(np.sqrt(DT))
SCAN_DT = "f16"            # "f16" or "f8" — low-precision dtype for the SDE scan

_prog_cache = {}


def _f(a):
    return np.ascontiguousarray(np.asarray(a, dtype=np.float64))


def _rep8(col):
    return np.tile(np.asarray(col, np.float32).reshape(-1, 1), (1, BPC))


def fold_params(params):
    """Host-side weight folding in float64, cast to on-chip dtypes."""
    w = {}
    s = 1.0 / np.sqrt(D)
    attn = params["attn"]
    w["wq"] = (_f(attn["q"]["w"]) * s).astype(np.float32)
    w["bq"] = (_f(attn["q"]["b"]) * s).astype(np.float32).reshape(D, 1)
    w["wk"] = _f(attn["k"]["w"]).astype(np.float32)
    w["bk"] = _f(attn["k"]["b"]).astype(np.float32).reshape(D, 1)
    w["wv"] = _f(attn["v"]["w"]).astype(np.float32)
    w["bv"] = _f(attn["v"]["b"]).astype(np.float32).reshape(D, 1)

    if SCAN_DT == "f8":
        import ml_dtypes
        np16 = ml_dtypes.float8_e4m3
    else:
        np16 = np.float16
    bias = {}
    for tag, net in (("d", params["drift"]), ("f", params["diff"])):
        win, bin_ = _f(net["in"]["w"]), _f(net["in"]["b"])
        w0, b0 = _f(net["fc"][0]["w"]), _f(net["fc"][0]["b"])
        w1, b1 = _f(net["fc"][1]["w"]), _f(net["fc"][1]["b"])
        wo, bo = _f(net["out"]["w"]), _f(net["out"]["b"])
        w[f"w{tag}1"] = (win @ w0).astype(np16)                        # 128x256
        w[f"w{tag}1t"] = (float(DT) * (win @ w0)).astype(np16)         # dt-scaled
        bias[f"{tag}1"] = (bin_ @ w0 + b0).astype(np.float32)
        w[f"w{tag}2"] = w1.astype(np16)                                # 256x256
        bias[f"{tag}2"] = b1.astype(np.float32)
        w[f"w{tag}3"] = wo.astype(np16)                                # 256x128
        bias[f"{tag}3"] = bo.astype(np.float32)
    # PSUM bias preload tiles: joint layout [d-m0 | d-m1 | f-m0 | f-m1] x 8 cols
    w["bw1"] = np.concatenate(
        [_rep8(bias["d1"][:128]), _rep8(bias["d1"][128:]),
         _rep8(bias["f1"][:128]), _rep8(bias["f1"][128:])], axis=1)
    w["bw2"] = np.concatenate(
        [_rep8(bias["d2"][:128]), _rep8(bias["d2"][128:]),
         _rep8(bias["f2"][:128]), _rep8(bias["f2"][128:])], axis=1)
    w["bw3"] = np.concatenate([_rep8(bias["f3"]), _rep8(bias["d3"])], axis=1)

    wfi, bfi = _f(params["fc_in"]["w"]), _f(params["fc_in"]["b"])
    wb0, bb0 = _f(params["fc_block"][0]["w"]), _f(params["fc_block"][0]["b"])
    wb1, bb1 = _f(params["fc_block"][1]["w"]), _f(params["fc_block"][1]["b"])
    wo, bo = _f(params["fc_out"]["w"]), _f(params["fc_out"]["b"])
    w["wu1"] = (wfi @ wb0).astype(np16)                                # 256x256
    w["bu1"] = (bfi @ wb0 + bb0).astype(np.float32).reshape(L, 1)
    w["wu2"] = wb1.astype(np16)                                        # 256x256
    w["bu2"] = bb1.astype(np.float32).reshape(L, 1)
    w["wu3"] = wo.astype(np16)                                         # 256x128
    w["bu3"] = bo.astype(np.float32).reshape(D, 1)
    return w


def build_program(n_steps=NSTEPS, do_scan=True, do_attn=True, debug=False,
                  scan_dt=None, attn_stride=40, FUSE_ILV=False):
    scan_dt = scan_dt or SCAN_DT
    import concourse.bacc as bacc
    import concourse.tile as tile
    from concourse import masks, mybir
    from contextlib import ExitStack

    f32 = mybir.dt.float32
    f16 = {"f16": mybir.dt.float16, "f8": mybir.dt.float8e4}[scan_dt]
    AF = mybir.ActivationFunctionType
    ALU = mybir.AluOpType

    Tloc = n_steps + 1

    nc = bacc.Bacc("TRN2", target_bir_lowering=False, debug=False)

    # ---- DRAM I/O ----
    x_d = nc.dram_tensor("x", (BPC, T, D), f32, kind="ExternalInput")
    nz_d = nc.dram_tensor("noise", (NSTEPS, BPC, H), f32, kind="ExternalInput")
    dram = {}
    for name, shape, dt in [
        ("wq", (D, D), f32), ("bq", (D, 1), f32),
        ("wk", (D, D), f32), ("bk", (D, 1), f32),
        ("wv", (D, D), f32), ("bv", (D, 1), f32),
        ("wd1", (H, L), f16), ("wd2", (L, L), f16), ("wd3", (L, H), f16),
        ("wf1", (H, L), f16), ("wf2", (L, L), f16), ("wf3", (L, H), f16),
        ("wd1t", (H, L), f16), ("wf1t", (H, L), f16),
        ("bw1", (128, 4 * BPC), f32), ("bw2", (128, 4 * BPC), f32),
        ("bw3", (128, 2 * BPC), f32),
        ("wu1", (L, L), f16), ("bu1", (L, 1), f32),
        ("wu2", (L, L), f16), ("bu2", (L, 1), f32),
        ("wu3", (L, D), f16), ("bu3", (D, 1), f32),
    ]:
        dram[name] = nc.dram_tensor(name, shape, dt, kind="ExternalInput")
    out_d = nc.dram_tensor("out", (BPC, T, D), f32, kind="ExternalOutput")
    if debug:
        hidT_d = nc.dram_tensor("dbg_hidT", (H, BPC, Tloc), f32, kind="ExternalOutput")
        ctxT_d = nc.dram_tensor("dbg_ctxT", (BPC, D, T), f32, kind="ExternalOutput")

    with tile.TileContext(nc) as tc, ExitStack() as octx:
        const = octx.enter_context(tc.tile_pool(name="const", bufs=1))

        ident = const.tile([128, 128], f32)
        masks.make_identity(nc, ident[:])

        sb = {}
        for name in ("wq", "wk", "wv"):
            sb[name] = const.tile([D, D], f32, name=name)
            nc.sync.dma_start(out=sb[name], in_=dram[name].ap())
        for name in ("bq", "bk", "bv", "bu3"):
            sb[name] = const.tile([128, 1], f32, name=name)
            nc.sync.dma_start(out=sb[name], in_=dram[name].ap())
        for name in ("bw1", "bw2", "bw3"):
            shape = [128, 4 * BPC] if name != "bw3" else [128, 2 * BPC]
            sb[name] = const.tile(shape, f32, name=name)
            nc.sync.dma_start(out=sb[name], in_=dram[name].ap())
        # SDE f16 weights; K>128 stored [128, kc, M]
        for tag in ("d", "f"):
            sb[f"w{tag}1"] = const.tile([H, L], f16, name=f"w{tag}1")
            nc.sync.dma_start(out=sb[f"w{tag}1"], in_=dram[f"w{tag}1"].ap())
            sb[f"w{tag}1t"] = const.tile([H, L], f16, name=f"w{tag}1t")
            nc.sync.dma_start(out=sb[f"w{tag}1t"], in_=dram[f"w{tag}1t"].ap())
            sb[f"w{tag}2"] = const.tile([128, 2, L], f16, name=f"w{tag}2")
            nc.sync.dma_start(
                out=sb[f"w{tag}2"],
                in_=dram[f"w{tag}2"].ap().rearrange("(c p) m -> p c m", p=128))
            sb[f"w{tag}3"] = const.tile([128, 2, H], f16, name=f"w{tag}3")
            nc.sync.dma_start(
                out=sb[f"w{tag}3"],
                in_=dram[f"w{tag}3"].ap().rearrange("(c p) m -> p c m", p=128))
        # fusion f32 weights
        for name in ("wu1", "wu2"):
            sb[name] = const.tile([128, 2, L], f16, name=name)
            nc.sync.dma_start(
                out=sb[name],
                in_=dram[name].ap().rearrange("(c p) m -> p c m", p=128))
        sb["wu3"] = const.tile([128, 2, D], f16, name="wu3")
        nc.sync.dma_start(
            out=sb["wu3"], in_=dram["wu3"].ap().rearrange("(c p) m -> p c m", p=128))
        for name in ("bu1", "bu2"):
            sb[name] = const.tile([128, 2], f32, name=name)
            nc.sync.dma_start(
                out=sb[name],
                in_=dram[name].ap().rearrange("(c p) one -> p (c one)", p=128))

        # persistent state; hidden split at t=SPLIT so the fusion's first
        # T-half has a clean whole-tile dependency (no mid-scan subtile races)
        SPLIT = T // 2 if n_steps == T - 1 else Tloc
        hid_a = const.tile([H, BPC, SPLIT], f32)
        hid_b = const.tile([H, BPC, max(Tloc - SPLIT, 1)], f32)

        def hid(c):
            return (hid_a[:, :, c] if c < SPLIT else hid_b[:, :, c - SPLIT])

        def hid_batch(b, half):
            return (hid_a if half == 0 else hid_b)[:, b, :]

        hid16_a = const.tile([H, BPC, SPLIT], f16)
        hid16_b = const.tile([H, BPC, max(Tloc - SPLIT, 1)], f16)

        def hid16(c):
            return (hid16_a[:, :, c] if c < SPLIT
                    else hid16_b[:, :, c - SPLIT])

        def hid16_batch(b, half):
            return (hid16_a if half == 0 else hid16_b)[:, b, :]
        noiseT = const.tile([H, n_steps * BPC], f32)  # sqrt(dt)*dw, feature-major
        ctxT_l = [const.tile([D, T], f16, name=f"ctxT{b}") for b in range(BPC)]

        # pools shared by scan + attention so they can interleave
        scn = octx.enter_context(tc.tile_pool(name="scn", bufs=2))
        scnp = octx.enter_context(tc.tile_pool(name="scnp", bufs=1, space="PSUM"))
        atp = octx.enter_context(tc.tile_pool(name="atp", bufs=2))
        apb = octx.enter_context(tc.tile_pool(name="apb", bufs=1, space="PSUM"))
        aps = octx.enter_context(tc.tile_pool(name="aps", bufs=1, space="PSUM"))
        ntp = octx.enter_context(tc.tile_pool(name="ntp", bufs=3))

        def emit_noise_block(t0):
            tcnt = min(16, n_steps - t0)
            rows = tcnt * BPC
            nz = ntp.tile([128, H], f32, tag="nz")
            nc.sync.dma_start(
                out=nz[:rows, :],
                in_=nz_d.ap()[t0:t0 + tcnt].rearrange("t b h -> (t b) h"))
            ps = aps.tile([128, 128], f32, tag="tps")
            nc.tensor.transpose(ps[:, :rows], nz[:rows, :], ident[:rows, :rows])
            nc.scalar.activation(
                out=noiseT[:, t0 * BPC: t0 * BPC + rows], in_=ps[:, :rows],
                func=AF.Copy, scale=float(SQDT))

        for t0 in range(0, min(48, n_steps), 16):
            emit_noise_block(t0)

        fusion_done = set()

        def emit_attn(b):
            """Attention part A for batch b (independent of the scan)."""
            xt = atp.tile([128, 4, D], f32, tag="xt")
            nc.sync.dma_start(
                out=xt, in_=x_d.ap()[b].rearrange("(c p) d -> p c d", p=128))
            xT = atp.tile([D, T], f32, tag="xT")
            for c in range(4):
                ps = aps.tile([128, 128], f32, tag="tps")
                nc.tensor.transpose(ps, xt[:, c, :], ident)
                nc.scalar.copy(xT[:, c * 128:(c + 1) * 128], ps)
            qT = atp.tile([D, T], f32, tag="qT")
            kT = atp.tile([D, T], f32, tag="kT")
            for wname, bname, dst in (("wq", "bq", qT), ("wk", "bk", kT)):
                ps = apb.tile([D, T], f32, tag="bps")
                nc.tensor.matmul(ps, lhsT=sb[wname], rhs=xT, start=True, stop=True)
                nc.scalar.activation(out=dst, in_=ps, func=AF.Identity,
                                     bias=sb[bname])
            v = atp.tile([128, 4, D], f32, tag="v")
            for c in range(4):
                ps = aps.tile([128, 128], f32, tag="tps")
                nc.tensor.matmul(ps, lhsT=xT[:, c * 128:(c + 1) * 128],
                                 rhs=sb["wv"], start=True, stop=True)
                nc.scalar.copy(v[:, c, :], ps)
            # scores (t-major) -> exp (+fused row-sum) -> normalize
            P = atp.tile([128, 4, T], f32, tag="P")
            sums = atp.tile([128, 4], f32, tag="sums")
            for c in range(4):
                ps = apb.tile([128, T], f32, tag="bps")
                nc.tensor.matmul(ps, lhsT=qT[:, c * 128:(c + 1) * 128],
                                 rhs=kT, start=True, stop=True)
                nc.scalar.activation(out=P[:, c, :], in_=ps, func=AF.Exp,
                                     accum_out=sums[:, c:c + 1])
            rec = atp.tile([128, 4], f32, tag="rec")
            nc.vector.reciprocal(rec, sums)
            for c in range(4):
                nc.vector.tensor_scalar_mul(P[:, c, :], P[:, c, :],
                                            rec[:, c:c + 1])
            # transpose P -> PT (s-major)
            PT = atp.tile([128, 4, T], f32, tag="PT")
            for tc4 in range(4):
                for sc in range(4):
                    ps = aps.tile([128, 128], f32, tag="tps")
                    nc.tensor.transpose(
                        ps, P[:, tc4, sc * 128:(sc + 1) * 128], ident)
                    nc.scalar.copy(PT[:, sc, tc4 * 128:(tc4 + 1) * 128], ps)
            # context^T = sum_s v[s,:]^T P^T[s,:] (+ bv)
            psc = apb.tile([D, T], f32, tag="bps")
            for sc in range(4):
                nc.tensor.matmul(psc, lhsT=v[:, sc, :], rhs=PT[:, sc, :],
                                 start=(sc == 0), stop=(sc == 3))
            nc.scalar.activation(out=ctxT_l[b], in_=psc, func=AF.Identity,
                                 bias=sb["bv"])
            if debug:
                nc.sync.dma_start(out=ctxT_d.ap()[b], in_=ctxT_l[b])

        TH = T // 2

        def emit_fusion(b, half):
            tlo = half * TH
            hu1 = atp.tile([128, 2, TH], f16, tag="hu1")
            for mi in range(2):
                ps = scnp.tile([128, TH], f32, tag="fps", bufs=2)
                nc.tensor.matmul(
                    ps, lhsT=sb["wu1"][:, 0, mi * 128:(mi + 1) * 128],
                    rhs=ctxT_l[b][:, tlo:tlo + TH], start=True, stop=False)
                nc.tensor.matmul(
                    ps, lhsT=sb["wu1"][:, 1, mi * 128:(mi + 1) * 128],
                    rhs=hid16_batch(b, half), start=False, stop=True)
                nc.vector.tensor_scalar(
                    out=hu1[:, mi, :], in0=ps, scalar1=sb["bu1"][:, mi:mi + 1],
                    scalar2=0.0, op0=ALU.add, op1=ALU.max)
            hu2 = atp.tile([128, 2, TH], f16, tag="hu2")
            for mi in range(2):
                ps = scnp.tile([128, TH], f32, tag="fps", bufs=2)
                nc.tensor.matmul(
                    ps, lhsT=sb["wu2"][:, 0, mi * 128:(mi + 1) * 128],
                    rhs=hu1[:, 0, :], start=True, stop=False)
                nc.tensor.matmul(
                    ps, lhsT=sb["wu2"][:, 1, mi * 128:(mi + 1) * 128],
                    rhs=hu1[:, 1, :], start=False, stop=True)
                nc.vector.tensor_scalar(
                    out=hu2[:, mi, :], in0=ps, scalar1=sb["bu2"][:, mi:mi + 1],
                    scalar2=0.0, op0=ALU.add, op1=ALU.max)
            pso = scnp.tile([D, TH], f32, tag="fps", bufs=2)
            nc.tensor.matmul(pso, lhsT=sb["wu3"][:, 0, :], rhs=hu2[:, 0, :],
                             start=True, stop=False)
            nc.tensor.matmul(pso, lhsT=sb["wu3"][:, 1, :], rhs=hu2[:, 1, :],
                             start=False, stop=True)
            outT = atp.tile([D, TH], f32, tag="outT")
            nc.scalar.activation(out=outT, in_=pso, func=AF.Identity,
                                 bias=sb["bu3"])
            ot = atp.tile([128, 2, D], f32, tag="ot")
            for c in range(2):
                ps = aps.tile([128, 128], f32, tag="tps")
                nc.tensor.transpose(ps, outT[:, c * 128:(c + 1) * 128], ident)
                nc.scalar.copy(ot[:, c, :], ps)
            nc.sync.dma_start(
                out=out_d.ap()[b][tlo:tlo + TH].rearrange("(c p) d -> p c d", p=128),
                in_=ot)

        # ---- phase 2: SDE scan (with attention part A interleaved) ----
        attn_queue = list(range(BPC)) if (do_attn and n_steps == T - 1) else []
        if do_scan:
            nc.vector.memset(hid_a[:, :, 0], 0.0)
            nc.vector.memset(hid16_a[:, :, 0], 0.0)
            y16 = hid16_a[:, :, 0]
            W1 = {t: sb[f"w{t}1"] for t in ("d", "f")}
            W2 = {t: sb[f"w{t}2"] for t in ("d", "f")}
            W3 = {t: sb[f"w{t}3"] for t in ("d", "f")}
            # fc1 is linear in the update, so p1(t+1) accumulates
            # bias + W1.y(t) (early) + dt.W1.sigd + W1.(sigf*dw) (late);
            # only the sigf*dw term is on the step's critical path.
            def fc1_accum(dst, rhs, suffix="", stop=False):
                for ni, tag in enumerate(("d", "f")):
                    for mi in range(2):
                        nc.tensor.matmul(
                            dst[:, (ni * 2 + mi) * BPC:(ni * 2 + mi + 1) * BPC],
                            lhsT=sb[f"w{tag}1{suffix}"][:, mi * 128:(mi + 1) * 128],
                            rhs=rhs, start=False, stop=stop,
                            skip_group_check=True)

            p1 = scnp.tile([128, 4 * BPC], f32, tag="pj1", bufs=2)
            nc.vector.tensor_copy(p1, sb["bw1"])
            fc1_accum(p1, y16, stop=True)
            for t in range(n_steps):
                if t % 16 == 0 and t + 48 < n_steps:
                    emit_noise_block(t + 48)
                if attn_queue and t == 24 + (BPC - len(attn_queue)) * attn_stride:
                    emit_attn(attn_queue.pop(0))
                if (FUSE_ILV and do_attn and n_steps == T - 1 and t >= 328
                        and t % 16 == 8 and len(fusion_done) < BPC):
                    b = len(fusion_done)
                    emit_fusion(b, 0)
                    fusion_done.add((b, 0))
                # start next step's p1 with the terms known early
                p1n = scnp.tile([128, 4 * BPC], f32, tag="pj1", bufs=2)
                nc.vector.tensor_copy(p1n, sb["bw1"])
                fc1_accum(p1n, y16)
                h1 = scn.tile([128, 4 * BPC], f16, tag="h1")
                nc.vector.tensor_scalar(out=h1, in0=p1, scalar1=0.0, scalar2=None,
                                        op0=ALU.max)
                p2 = scnp.tile([128, 4 * BPC], f32, tag="pj2", bufs=2)
                nc.scalar.copy(p2, sb["bw2"])
                for ni, tag in enumerate(("d", "f")):
                    for mi in range(2):
                        reg = p2[:, (ni * 2 + mi) * BPC:(ni * 2 + mi + 1) * BPC]
                        nc.tensor.matmul(
                            reg, lhsT=W2[tag][:, 0, mi * 128:(mi + 1) * 128],
                            rhs=h1[:, ni * 2 * BPC:(ni * 2 + 1) * BPC],
                            start=False, stop=False, skip_group_check=True)
                        nc.tensor.matmul(
                            reg, lhsT=W2[tag][:, 1, mi * 128:(mi + 1) * 128],
                            rhs=h1[:, (ni * 2 + 1) * BPC:(ni * 2 + 2) * BPC],
                            start=False, stop=True, skip_group_check=True)
                h2 = scn.tile([128, 4 * BPC], f16, tag="h2")
                nc.vector.tensor_scalar(out=h2, in0=p2, scalar1=0.0, scalar2=None,
                                        op0=ALU.max)
                # p3 joint [f | d]: diff first so the noise-multiply
                # (which gates y16) starts as early as possible
                p3 = scnp.tile([H, 2 * BPC], f32, tag="pj2", bufs=2)
                nc.scalar.copy(p3, sb["bw3"])
                hslc = {"d": (h2[:, 0:BPC], h2[:, BPC:2 * BPC]),
                        "f": (h2[:, 2 * BPC:3 * BPC], h2[:, 3 * BPC:4 * BPC])}
                for ni, tag in enumerate(("f", "d")):
                    reg = p3[:, ni * BPC:(ni + 1) * BPC]
                    nc.tensor.matmul(
                        reg, lhsT=W3[tag][:, 0, :], rhs=hslc[tag][0],
                        start=False, stop=False, skip_group_check=True)
                    nc.tensor.matmul(
                        reg, lhsT=W3[tag][:, 1, :], rhs=hslc[tag][1],
                        start=False, stop=True, skip_group_check=True)
                sg = scn.tile([H, 2 * BPC], f32, tag="sg")
                nc.scalar.activation(out=sg, in_=p3, func=AF.Sigmoid)
                # critical path: fp16 delta terms feed next step's fc1 PSUM
                ftmp16 = scn.tile([H, BPC], f16, tag="ftmp16")
                nc.vector.tensor_mul(
                    ftmp16, sg[:, 0:BPC], noiseT[:, t * BPC:(t + 1) * BPC])
                sgd16 = scn.tile([H, BPC], f16, tag="sgd16")
                nc.vector.tensor_copy(sgd16, sg[:, BPC:2 * BPC])
                for ni, tag in enumerate(("d", "f")):
                    for mi in range(2):
                        sl = p1n[:, (ni * 2 + mi) * BPC:(ni * 2 + mi + 1) * BPC]
                        nc.tensor.matmul(
                            sl, lhsT=sb[f"w{tag}1t"][:, mi * 128:(mi + 1) * 128],
                            rhs=sgd16, start=False, stop=False,
                            skip_group_check=True)
                        nc.tensor.matmul(
                            sl, lhsT=sb[f"w{tag}1"][:, mi * 128:(mi + 1) * 128],
                            rhs=ftmp16, start=False, stop=True,
                            skip_group_check=True)
                # off critical path: exact fp32 state update
                ftmp = scn.tile([H, BPC], f32, tag="ftmp")
                nc.gpsimd.tensor_mul(
                    ftmp, sg[:, 0:BPC], noiseT[:, t * BPC:(t + 1) * BPC])
                ytmp = scn.tile([H, BPC], f32, tag="ytmp")
                nc.vector.scalar_tensor_tensor(
                    out=ytmp, in0=sg[:, BPC:2 * BPC], scalar=float(DT),
                    in1=hid(t), op0=ALU.mult, op1=ALU.add)
                nc.gpsimd.tensor_add(hid(t + 1), ytmp, ftmp)
                y16 = hid16(t + 1)
                nc.vector.tensor_add(y16, ytmp, ftmp)
                p1 = p1n
        else:
            nc.vector.memset(hid_a[:, :, :], 0.0)
            nc.vector.memset(hid_b[:, :, :], 0.0)
        for b in attn_queue:
            emit_attn(b)
        if debug:
            nc.sync.dma_start(out=hidT_d.ap()[:, :, 0:SPLIT], in_=hid_a)
            if Tloc > SPLIT:
                nc.sync.dma_start(out=hidT_d.ap()[:, :, SPLIT:], in_=hid_b)

        # ---- phase 3 epilogue: remaining fusion halves ----
        if do_attn:
            assert Tloc == T
            for b in range(BPC):
                if (b, 0) not in fusion_done:
                    emit_fusion(b, 0)
                emit_fusion(b, 1)
    nc.compile()
    return nc


def _get_program():
    key = ("full", SCAN_DT)
    if key not in _prog_cache:
        _prog_cache[key] = build_program()
    return _prog_cache[key]


def kernel(x, noise, params):
    from concourse import bass_utils

    x = np.ascontiguousarray(np.asarray(x, dtype=np.float32))
    noise = np.ascontiguousarray(np.asarray(noise, dtype=np.float32))
    w = fold_params(params)

    nc = _get_program()
    in_maps = []
    for c in range(NCORES):
        m = dict(w)
        m["x"] = np.ascontiguousarray(x[c * BPC:(c + 1) * BPC])
        m["noise"] = np.ascontiguousarray(noise[:, c * BPC:(c + 1) * BPC])
        in_maps.append(m)
    res = bass_utils.run_bass_kernel_spmd(nc, in_maps, core_ids=list(range(NCORES)))
    return np.concatenate([res.results[c]["out"] for c in range(NCORES)], axis=0)


if __name__ == "__main__":
    import pickle, time
    x = np.load("/tmp/x.npy")
    noise = np.load("/tmp/noise.npy")
    with open("/tmp/params.pkl", "rb") as f:
        params = pickle.load(f)
    t0 = time.time()
    out = kernel(x, noise, params)
    print("kernel() wall:", time.time() - t0)
    ref = np.load("/tmp/ref_np.npy")
    err = np.abs(out - ref)
    print("absmax:", err.max(), "rel:", err.max() / np.abs(ref).max())


# revision 25
# speedup vs baseline: 1.1035x; 1.0591x over previous
"""Trainium2 Bass kernel for AttentionAugmentedNSDE.

Model (B=64, T=512, D=H=128, L=256):
  1. single-head scaled-dot-product self-attention over x (B,T,D)
  2. Euler-Maruyama neural SDE over T-1 steps: y' = y + drift(y)*dt + diff(y)*(sqrt(dt)*dw)
     drift/diff: Linear(128->256) -> [Linear(256,256)+ReLU]*2 -> Linear(256->128)+Sigmoid
  3. fusion MLP on concat([context, hidden]) per token.

Strategy:
  - Data-parallel over batch: B=64 -> 8 cores x 8 samples, no collectives.
  - Algebraic folds (host, float64): the first Linear of each SDE net has no
    activation so W_in@W_fc0 folds into one 128->256 layer; same for
    fc_in@fc_block[0] in the fusion MLP; attention 1/sqrt(D) folded into Wq;
    v-bias folded to after-context (softmax rows sum to 1).
  - Feature-major layout on chip (features on partitions, batch/time on free
    dim); weights are the stationary matmul operand.
  - SDE scan in fp16 (weights+activations; fp32 PSUM accumulation + fp32
    state). The scan is latency-bound on the per-step dependency chain, so
    both nets share joint PSUM tiles (one elementwise op per layer), biases
    are preloaded into PSUM off the critical path, and the attention phase is
    interleaved into the scan to fill engine gaps and keep the PE clock warm.
"""

import sys

import numpy as np

_TRN = "/opt/trn_rl_repo"
if _TRN not in sys.path:
    sys.path.insert(0, _TRN)

B, T, D, H, L = 64, 512, 128, 128, 256
NCORES = 8
BPC = B // NCORES          # batches per core
NSTEPS = T - 1             # 511 scan steps
DT = np.float32(1.0 / NSTEPS)
SQDT = np.float32(np.sqrt(DT))
SCAN_DT = "f16"            # "f16" or "f8" — low-precision dtype for the SDE scan

_prog_cache = {}


def _f(a):
    return np.ascontiguousarray(np.asarray(a, dtype=np.float64))


def _rep8(col):
    return np.tile(np.asarray(col, np.float32).reshape(-1, 1), (1, BPC))


def fold_params(params):
    """Host-side weight folding in float64, cast to on-chip dtypes."""
    w = {}
    s = 1.0 / np.sqrt(D)
    attn = params["attn"]
    w["wq"] = (_f(attn["q"]["w"]) * s).astype(np.float32)
    w["bq"] = (_f(attn["q"]["b"]) * s).astype(np.float32).reshape(D, 1)
    w["wk"] = _f(attn["k"]["w"]).astype(np.float32)
    w["bk"] = _f(attn["k"]["b"]).astype(np.float32).reshape(D, 1)
    w["wv"] = _f(attn["v"]["w"]).astype(np.float32)
    w["bv"] = _f(attn["v"]["b"]).astype(np.float32).reshape(D, 1)

    if SCAN_DT == "f8":
        import ml_dtypes
        np16 = ml_dtypes.float8_e4m3
    else:
        np16 = np.float16
    bias = {}
    for tag, net in (("d", params["drift"]), ("f", params["diff"])):
        win, bin_ = _f(net["in"]["w"]), _f(net["in"]["b"])
        w0, b0 = _f(net["fc"][0]["w"]), _f(net["fc"][0]["b"])
        w1, b1 = _f(net["fc"][1]["w"]), _f(net["fc"][1]["b"])
        wo, bo = _f(net["out"]["w"]), _f(net["out"]["b"])
        w[f"w{tag}1"] = (win @ w0).astype(np16)                        # 128x256
        w[f"w{tag}1t"] = (float(DT) * (win @ w0)).astype(np16)         # dt-scaled
        bias[f"{tag}1"] = (bin_ @ w0 + b0).astype(np.float32)
        w[f"w{tag}2"] = w1.astype(np16)                                # 256x256
        bias[f"{tag}2"] = b1.astype(np.float32)
        w[f"w{tag}3"] = wo.astype(np16)                                # 256x128
        bias[f"{tag}3"] = bo.astype(np.float32)
    # PSUM bias preload tiles: joint layout [d-m0 | d-m1 | f-m0 | f-m1] x 8 cols
    w["bw1"] = np.concatenate(
        [_rep8(bias["d1"][:128]), _rep8(bias["d1"][128:]),
         _rep8(bias["f1"][:128]), _rep8(bias["f1"][128:])], axis=1)
    w["bw2"] = np.concatenate(
        [_rep8(bias["d2"][:128]), _rep8(bias["d2"][128:]),
         _rep8(bias["f2"][:128]), _rep8(bias["f2"][128:])], axis=1)
    w["bw3"] = np.concatenate([_rep8(bias["f3"]), _rep8(bias["d3"])], axis=1)

    wfi, bfi = _f(params["fc_in"]["w"]), _f(params["fc_in"]["b"])
    wb0, bb0 = _f(params["fc_block"][0]["w"]), _f(params["fc_block"][0]["b"])
    wb1, bb1 = _f(params["fc_block"][1]["w"]), _f(params["fc_block"][1]["b"])
    wo, bo = _f(params["fc_out"]["w"]), _f(params["fc_out"]["b"])
    w["wu1"] = (wfi @ wb0).astype(np16)                                # 256x256
    w["bu1"] = (bfi @ wb0 + bb0).astype(np.float32).reshape(L, 1)
    w["wu2"] = wb1.astype(np16)                                        # 256x256
    w["bu2"] = bb1.astype(np.float32).reshape(L, 1)
    w["wu3"] = wo.astype(np16)                                         # 256x128
    w["bu3"] = bo.astype(np.float32).reshape(D, 1)
    return w


def build_program(n_steps=NSTEPS, do_scan=True, do_attn=True, debug=False,
                  scan_dt=None, attn_stride=56, FUSE_ILV=False):
    scan_dt = scan_dt or SCAN_DT
    import concourse.bacc as bacc
    import concourse.tile as tile
    from concourse import masks, mybir
    from contextlib import ExitStack

    f32 = mybir.dt.float32
    f16 = {"f16": mybir.dt.float16, "f8": mybir.dt.float8e4}[scan_dt]
    AF = mybir.ActivationFunctionType
    ALU = mybir.AluOpType

    Tloc = n_steps + 1

    nc = bacc.Bacc("TRN2", target_bir_lowering=False, debug=False)

    # ---- DRAM I/O ----
    x_d = nc.dram_tensor("x", (BPC, T, D), f32, kind="ExternalInput")
    nz_d = nc.dram_tensor("noise", (NSTEPS, BPC, H), f32, kind="ExternalInput")
    dram = {}
    for name, shape, dt in [
        ("wq", (D, D), f32), ("bq", (D, 1), f32),
        ("wk", (D, D), f32), ("bk", (D, 1), f32),
        ("wv", (D, D), f32), ("bv", (D, 1), f32),
        ("wd1", (H, L), f16), ("wd2", (L, L), f16), ("wd3", (L, H), f16),
        ("wf1", (H, L), f16), ("wf2", (L, L), f16), ("wf3", (L, H), f16),
        ("wd1t", (H, L), f16), ("wf1t", (H, L), f16),
        ("bw1", (128, 4 * BPC), f32), ("bw2", (128, 4 * BPC), f32),
        ("bw3", (128, 2 * BPC), f32),
        ("wu1", (L, L), f16), ("bu1", (L, 1), f32),
        ("wu2", (L, L), f16), ("bu2", (L, 1), f32),
        ("wu3", (L, D), f16), ("bu3", (D, 1), f32),
    ]:
        dram[name] = nc.dram_tensor(name, shape, dt, kind="ExternalInput")
    out_d = nc.dram_tensor("out", (BPC, T, D), f32, kind="ExternalOutput")
    if debug:
        hidT_d = nc.dram_tensor("dbg_hidT", (H, BPC, Tloc), f32, kind="ExternalOutput")
        ctxT_d = nc.dram_tensor("dbg_ctxT", (BPC, D, T), f32, kind="ExternalOutput")

    with tile.TileContext(nc) as tc, ExitStack() as octx:
        const = octx.enter_context(tc.tile_pool(name="const", bufs=1))

        ident = const.tile([128, 128], f32)
        masks.make_identity(nc, ident[:])

        sb = {}
        for name in ("wq", "wk", "wv"):
            sb[name] = const.tile([D, D], f32, name=name)
            nc.sync.dma_start(out=sb[name], in_=dram[name].ap())
        for name in ("bq", "bk", "bv", "bu3"):
            sb[name] = const.tile([128, 1], f32, name=name)
            nc.sync.dma_start(out=sb[name], in_=dram[name].ap())
        for name in ("bw1", "bw2", "bw3"):
            shape = [128, 4 * BPC] if name != "bw3" else [128, 2 * BPC]
            sb[name] = const.tile(shape, f32, name=name)
            nc.sync.dma_start(out=sb[name], in_=dram[name].ap())
        # SDE f16 weights; K>128 stored [128, kc, M]
        for tag in ("d", "f"):
            sb[f"w{tag}1"] = const.tile([H, L], f16, name=f"w{tag}1")
            nc.sync.dma_start(out=sb[f"w{tag}1"], in_=dram[f"w{tag}1"].ap())
            sb[f"w{tag}1t"] = const.tile([H, L], f16, name=f"w{tag}1t")
            nc.sync.dma_start(out=sb[f"w{tag}1t"], in_=dram[f"w{tag}1t"].ap())
            sb[f"w{tag}2"] = const.tile([128, 2, L], f16, name=f"w{tag}2")
            nc.sync.dma_start(
                out=sb[f"w{tag}2"],
                in_=dram[f"w{tag}2"].ap().rearrange("(c p) m -> p c m", p=128))
            sb[f"w{tag}3"] = const.tile([128, 2, H], f16, name=f"w{tag}3")
            nc.sync.dma_start(
                out=sb[f"w{tag}3"],
                in_=dram[f"w{tag}3"].ap().rearrange("(c p) m -> p c m", p=128))
        # fusion f32 weights
        for name in ("wu1", "wu2"):
            sb[name] = const.tile([128, 2, L], f16, name=name)
            nc.sync.dma_start(
                out=sb[name],
                in_=dram[name].ap().rearrange("(c p) m -> p c m", p=128))
        sb["wu3"] = const.tile([128, 2, D], f16, name="wu3")
        nc.sync.dma_start(
            out=sb["wu3"], in_=dram["wu3"].ap().rearrange("(c p) m -> p c m", p=128))
        for name in ("bu1", "bu2"):
            sb[name] = const.tile([128, 2], f32, name=name)
            nc.sync.dma_start(
                out=sb[name],
                in_=dram[name].ap().rearrange("(c p) one -> p (c one)", p=128))

        # persistent state
        hidden = const.tile([H, BPC, Tloc], f32)   # hidden^T, batch-major cols
        hidden16 = const.tile([H, BPC, Tloc], f16)  # fp16 copy for the fusion
        noiseT = const.tile([H, n_steps * BPC], f32)  # sqrt(dt)*dw, feature-major
        ctxT_l = [const.tile([D, T], f16, name=f"ctxT{b}") for b in range(BPC)]

        # pools shared by scan + attention so they can interleave
        scn = octx.enter_context(tc.tile_pool(name="scn", bufs=2))
        scnp = octx.enter_context(tc.tile_pool(name="scnp", bufs=1, space="PSUM"))
        atp = octx.enter_context(tc.tile_pool(name="atp", bufs=2))
        apb = octx.enter_context(tc.tile_pool(name="apb", bufs=1, space="PSUM"))
        aps = octx.enter_context(tc.tile_pool(name="aps", bufs=1, space="PSUM"))
        ntp = octx.enter_context(tc.tile_pool(name="ntp", bufs=3))

        def emit_noise_block(t0):
            tcnt = min(16, n_steps - t0)
            rows = tcnt * BPC
            nz = ntp.tile([128, H], f32, tag="nz")
            nc.sync.dma_start(
                out=nz[:rows, :],
                in_=nz_d.ap()[t0:t0 + tcnt].rearrange("t b h -> (t b) h"))
            ps = aps.tile([128, 128], f32, tag="tps")
            nc.tensor.transpose(ps[:, :rows], nz[:rows, :], ident[:rows, :rows])
            nc.scalar.activation(
                out=noiseT[:, t0 * BPC: t0 * BPC + rows], in_=ps[:, :rows],
                func=AF.Copy, scale=float(SQDT))

        for t0 in range(0, min(48, n_steps), 16):
            emit_noise_block(t0)

        fusion_done = set()

        def emit_attn(b):
            """Attention part A for batch b (independent of the scan)."""
            xt = atp.tile([128, 4, D], f32, tag="xt")
            nc.sync.dma_start(
                out=xt, in_=x_d.ap()[b].rearrange("(c p) d -> p c d", p=128))
            xT = atp.tile([D, T], f32, tag="xT")
            for c in range(4):
                ps = aps.tile([128, 128], f32, tag="tps")
                nc.tensor.transpose(ps, xt[:, c, :], ident)
                nc.scalar.copy(xT[:, c * 128:(c + 1) * 128], ps)
            qT = atp.tile([D, T], f32, tag="qT")
            kT = atp.tile([D, T], f32, tag="kT")
            for wname, bname, dst in (("wq", "bq", qT), ("wk", "bk", kT)):
                ps = apb.tile([D, T], f32, tag="bps")
                nc.tensor.matmul(ps, lhsT=sb[wname], rhs=xT, start=True, stop=True)
                nc.scalar.activation(out=dst, in_=ps, func=AF.Identity,
                                     bias=sb[bname])
            v = atp.tile([128, 4, D], f32, tag="v")
            for c in range(4):
                ps = aps.tile([128, 128], f32, tag="tps")
                nc.tensor.matmul(ps, lhsT=xT[:, c * 128:(c + 1) * 128],
                                 rhs=sb["wv"], start=True, stop=True)
                nc.scalar.copy(v[:, c, :], ps)
            # scores (t-major) -> exp (+fused row-sum) -> normalize
            P = atp.tile([128, 4, T], f32, tag="P")
            sums = atp.tile([128, 4], f32, tag="sums")
            for c in range(4):
                ps = apb.tile([128, T], f32, tag="bps")
                nc.tensor.matmul(ps, lhsT=qT[:, c * 128:(c + 1) * 128],
                                 rhs=kT, start=True, stop=True)
                nc.scalar.activation(out=P[:, c, :], in_=ps, func=AF.Exp,
                                     accum_out=sums[:, c:c + 1])
            rec = atp.tile([128, 4], f32, tag="rec")
            nc.vector.reciprocal(rec, sums)
            for c in range(4):
                nc.vector.tensor_scalar_mul(P[:, c, :], P[:, c, :],
                                            rec[:, c:c + 1])
            # transpose P -> PT (s-major)
            PT = atp.tile([128, 4, T], f32, tag="PT")
            for tc4 in range(4):
                for sc in range(4):
                    ps = aps.tile([128, 128], f32, tag="tps")
                    nc.tensor.transpose(
                        ps, P[:, tc4, sc * 128:(sc + 1) * 128], ident)
                    nc.scalar.copy(PT[:, sc, tc4 * 128:(tc4 + 1) * 128], ps)
            # context^T = sum_s v[s,:]^T P^T[s,:] (+ bv)
            psc = apb.tile([D, T], f32, tag="bps")
            for sc in range(4):
                nc.tensor.matmul(psc, lhsT=v[:, sc, :], rhs=PT[:, sc, :],
                                 start=(sc == 0), stop=(sc == 3))
            nc.scalar.activation(out=ctxT_l[b], in_=psc, func=AF.Identity,
                                 bias=sb["bv"])
            if debug:
                nc.sync.dma_start(out=ctxT_d.ap()[b], in_=ctxT_l[b])

        TH = T

        def emit_fusion(b, half):
            tlo = half * TH
            hu1 = atp.tile([128, 2, TH], f16, tag="hu1")
            for mi in range(2):
                ps = scnp.tile([128, TH], f32, tag="pj1", bufs=2)
                nc.tensor.matmul(
                    ps, lhsT=sb["wu1"][:, 0, mi * 128:(mi + 1) * 128],
                    rhs=ctxT_l[b][:, tlo:tlo + TH], start=True, stop=False)
                nc.tensor.matmul(
                    ps, lhsT=sb["wu1"][:, 1, mi * 128:(mi + 1) * 128],
                    rhs=hidden16[:, b, tlo:tlo + TH], start=False, stop=True)
                nc.vector.tensor_scalar(
                    out=hu1[:, mi, :], in0=ps, scalar1=sb["bu1"][:, mi:mi + 1],
                    scalar2=0.0, op0=ALU.add, op1=ALU.max)
            hu2 = atp.tile([128, 2, TH], f16, tag="hu2")
            for mi in range(2):
                ps = scnp.tile([128, TH], f32, tag="pj2", bufs=2)
                nc.tensor.matmul(
                    ps, lhsT=sb["wu2"][:, 0, mi * 128:(mi + 1) * 128],
                    rhs=hu1[:, 0, :], start=True, stop=False)
                nc.tensor.matmul(
                    ps, lhsT=sb["wu2"][:, 1, mi * 128:(mi + 1) * 128],
                    rhs=hu1[:, 1, :], start=False, stop=True)
                nc.vector.tensor_scalar(
                    out=hu2[:, mi, :], in0=ps, scalar1=sb["bu2"][:, mi:mi + 1],
                    scalar2=0.0, op0=ALU.add, op1=ALU.max)
            pso = scnp.tile([D, TH], f32, tag="pj3", bufs=2)
            nc.tensor.matmul(pso, lhsT=sb["wu3"][:, 0, :], rhs=hu2[:, 0, :],
                             start=True, stop=False)
            nc.tensor.matmul(pso, lhsT=sb["wu3"][:, 1, :], rhs=hu2[:, 1, :],
                             start=False, stop=True)
            outT = atp.tile([D, TH], f32, tag="outT")
            nc.scalar.activation(out=outT, in_=pso, func=AF.Identity,
                                 bias=sb["bu3"])
            ot = atp.tile([128, TH // 128, D], f32, tag="ot")
            for c in range(TH // 128):
                ps = aps.tile([128, 128], f32, tag="tps")
                nc.tensor.transpose(ps, outT[:, c * 128:(c + 1) * 128], ident)
                nc.scalar.copy(ot[:, c, :], ps)
            nc.sync.dma_start(
                out=out_d.ap()[b][tlo:tlo + TH].rearrange("(c p) d -> p c d", p=128),
                in_=ot)

        # ---- phase 2: SDE scan (with attention part A interleaved) ----
        attn_queue = list(range(BPC)) if (do_attn and n_steps == T - 1) else []
        if do_scan:
            nc.vector.memset(hidden[:, :, 0], 0.0)
            y16_0 = scn.tile([H, BPC], f16, tag="y16")
            nc.vector.memset(y16_0, 0.0)
            y16 = y16_0
            W1 = {t: sb[f"w{t}1"] for t in ("d", "f")}
            W2 = {t: sb[f"w{t}2"] for t in ("d", "f")}
            W3 = {t: sb[f"w{t}3"] for t in ("d", "f")}
            # fc1 is linear in the update, so p1(t+1) accumulates
            # bias + W1.y(t) (early) + dt.W1.sigd + W1.(sigf*dw) (late);
            # only the sigf*dw term is on the step's critical path.
            def fc1_accum(dst, rhs, suffix="", stop=False):
                for ni, tag in enumerate(("d", "f")):
                    for mi in range(2):
                        nc.tensor.matmul(
                            dst[:, (ni * 2 + mi) * BPC:(ni * 2 + mi + 1) * BPC],
                            lhsT=sb[f"w{tag}1{suffix}"][:, mi * 128:(mi + 1) * 128],
                            rhs=rhs, start=False, stop=stop,
                            skip_group_check=True)

            p1 = scnp.tile([128, 4 * BPC], f32, tag="pj1", bufs=2)
            nc.vector.tensor_copy(p1, sb["bw1"])
            fc1_accum(p1, y16, stop=True)
            for t in range(n_steps):
                if t % 16 == 0 and t + 48 < n_steps:
                    emit_noise_block(t + 48)
                if attn_queue and t == 24 + (BPC - len(attn_queue)) * attn_stride:
                    emit_attn(attn_queue.pop(0))
                if (FUSE_ILV and do_attn and n_steps == T - 1 and t >= 328
                        and t % 16 == 8 and len(fusion_done) < BPC):
                    b = len(fusion_done)
                    emit_fusion(b, 0)
                    fusion_done.add((b, 0))
                # start next step's p1 with the terms known early
                p1n = scnp.tile([128, 4 * BPC], f32, tag="pj1", bufs=2)
                nc.vector.tensor_copy(p1n, sb["bw1"])
                fc1_accum(p1n, y16)
                h1 = scn.tile([128, 4 * BPC], f16, tag="h1")
                nc.vector.tensor_scalar(out=h1, in0=p1, scalar1=0.0, scalar2=None,
                                        op0=ALU.max)
                p2 = scnp.tile([128, 4 * BPC], f32, tag="pj2", bufs=2)
                nc.scalar.copy(p2, sb["bw2"])
                for ni, tag in enumerate(("d", "f")):
                    for mi in range(2):
                        reg = p2[:, (ni * 2 + mi) * BPC:(ni * 2 + mi + 1) * BPC]
                        nc.tensor.matmul(
                            reg, lhsT=W2[tag][:, 0, mi * 128:(mi + 1) * 128],
                            rhs=h1[:, ni * 2 * BPC:(ni * 2 + 1) * BPC],
                            start=False, stop=False, skip_group_check=True)
                        nc.tensor.matmul(
                            reg, lhsT=W2[tag][:, 1, mi * 128:(mi + 1) * 128],
                            rhs=h1[:, (ni * 2 + 1) * BPC:(ni * 2 + 2) * BPC],
                            start=False, stop=True, skip_group_check=True)
                h2 = scn.tile([128, 4 * BPC], f16, tag="h2")
                nc.vector.tensor_scalar(out=h2, in0=p2, scalar1=0.0, scalar2=None,
                                        op0=ALU.max)
                # p3 joint [f | d]: diff first so the noise-multiply
                # (which gates y16) starts as early as possible
                p3 = scnp.tile([H, 2 * BPC], f32, tag="pj3", bufs=2)
                nc.scalar.copy(p3, sb["bw3"])
                hslc = {"d": (h2[:, 0:BPC], h2[:, BPC:2 * BPC]),
                        "f": (h2[:, 2 * BPC:3 * BPC], h2[:, 3 * BPC:4 * BPC])}
                for ni, tag in enumerate(("f", "d")):
                    reg = p3[:, ni * BPC:(ni + 1) * BPC]
                    nc.tensor.matmul(
                        reg, lhsT=W3[tag][:, 0, :], rhs=hslc[tag][0],
                        start=False, stop=False, skip_group_check=True)
                    nc.tensor.matmul(
                        reg, lhsT=W3[tag][:, 1, :], rhs=hslc[tag][1],
                        start=False, stop=True, skip_group_check=True)
                sg = scn.tile([H, 2 * BPC], f32, tag="sg")
                nc.scalar.activation(out=sg, in_=p3, func=AF.Sigmoid)
                # critical path: fp16 delta terms feed next step's fc1 PSUM
                ftmp16 = scn.tile([H, BPC], f16, tag="ftmp16")
                nc.vector.tensor_mul(
                    ftmp16, sg[:, 0:BPC], noiseT[:, t * BPC:(t + 1) * BPC])
                sgd16 = scn.tile([H, BPC], f16, tag="sgd16")
                nc.vector.tensor_copy(sgd16, sg[:, BPC:2 * BPC])
                for ni, tag in enumerate(("d", "f")):
                    for mi in range(2):
                        sl = p1n[:, (ni * 2 + mi) * BPC:(ni * 2 + mi + 1) * BPC]
                        nc.tensor.matmul(
                            sl, lhsT=sb[f"w{tag}1t"][:, mi * 128:(mi + 1) * 128],
                            rhs=sgd16, start=False, stop=False,
                            skip_group_check=True)
                        nc.tensor.matmul(
                            sl, lhsT=sb[f"w{tag}1"][:, mi * 128:(mi + 1) * 128],
                            rhs=ftmp16, start=False, stop=True,
                            skip_group_check=True)
                # off critical path: exact fp32 state update
                ftmp = scn.tile([H, BPC], f32, tag="ftmp")
                nc.gpsimd.tensor_mul(
                    ftmp, sg[:, 0:BPC], noiseT[:, t * BPC:(t + 1) * BPC])
                ytmp = scn.tile([H, BPC], f32, tag="ytmp")
                nc.vector.scalar_tensor_tensor(
                    out=ytmp, in0=sg[:, BPC:2 * BPC], scalar=float(DT),
                    in1=hidden[:, :, t], op0=ALU.mult, op1=ALU.add)
                nc.gpsimd.tensor_add(hidden[:, :, t + 1], ytmp, ftmp)
                y16 = scn.tile([H, BPC], f16, tag="y16")
                nc.vector.tensor_add(y16, ytmp, ftmp)
                p1 = p1n
        else:
            nc.vector.memset(hidden[:, :, :], 0.0)
        for b in attn_queue:
            emit_attn(b)
        if debug:
            nc.sync.dma_start(out=hidT_d.ap(), in_=hidden)

        # fp16 copy of hidden for the fusion matmuls
        if do_attn:
            half_cols = BPC * Tloc // 2
            hv32 = hidden.rearrange("p b t -> p (b t)")
            hv16 = hidden16.rearrange("p b t -> p (b t)")
            nc.vector.tensor_copy(hv16[:, 0:half_cols], hv32[:, 0:half_cols])
            nc.vector.tensor_copy(hv16[:, half_cols:], hv32[:, half_cols:])

        # ---- phase 3 epilogue: remaining fusion halves ----
        if do_attn:
            assert Tloc == T
            for b in range(BPC):
                emit_fusion(b, 0)
    nc.compile()
    return nc


def _get_program():
    key = ("full", SCAN_DT)
    if key not in _prog_cache:
        _prog_cache[key] = build_program()
    return _prog_cache[key]


def kernel(x, noise, params):
    from concourse import bass_utils

    x = np.ascontiguousarray(np.asarray(x, dtype=np.float32))
    noise = np.ascontiguousarray(np.asarray(noise, dtype=np.float32))
    w = fold_params(params)

    nc = _get_program()
    in_maps = []
    for c in range(NCORES):
        m = dict(w)
        m["x"] = np.ascontiguousarray(x[c * BPC:(c + 1) * BPC])
        m["noise"] = np.ascontiguousarray(noise[:, c * BPC:(c + 1) * BPC])
        in_maps.append(m)
    res = bass_utils.run_bass_kernel_spmd(nc, in_maps, core_ids=list(range(NCORES)))
    return np.concatenate([res.results[c]["out"] for c in range(NCORES)], axis=0)


if __name__ == "__main__":
    import pickle, time
    x = np.load("/tmp/x.npy")
    noise = np.load("/tmp/noise.npy")
    with open("/tmp/params.pkl", "rb") as f:
        params = pickle.load(f)
    t0 = time.time()
    out = kernel(x, noise, params)
    print("kernel() wall:", time.time() - t0)
    ref = np.load("/tmp/ref_np.npy")
    err = np.abs(out - ref)
    print("absmax:", err.max(), "rel:", err.max() / np.abs(ref).max())


# revision 26
# speedup vs baseline: 1.3643x; 1.2364x over previous
"""Trainium2 Bass kernel for AttentionAugmentedNSDE.

Model (B=64, T=512, D=H=128, L=256):
  1. single-head scaled-dot-product self-attention over x (B,T,D)
  2. Euler-Maruyama neural SDE over T-1 steps: y' = y + drift(y)*dt + diff(y)*(sqrt(dt)*dw)
     drift/diff: Linear(128->256) -> [Linear(256,256)+ReLU]*2 -> Linear(256->128)+Sigmoid
  3. fusion MLP on concat([context, hidden]) per token.

Strategy:
  - Data-parallel over batch: B=64 -> 8 cores x 8 samples, no collectives.
  - Algebraic folds (host, float64): the first Linear of each SDE net has no
    activation so W_in@W_fc0 folds into one 128->256 layer; same for
    fc_in@fc_block[0] in the fusion MLP; attention 1/sqrt(D) folded into Wq;
    v-bias folded to after-context (softmax rows sum to 1).
  - Feature-major layout on chip (features on partitions, batch/time on free
    dim); weights are the stationary matmul operand.
  - SDE scan in fp16 (weights+activations; fp32 PSUM accumulation + fp32
    state). The scan is latency-bound on the per-step dependency chain, so
    both nets share joint PSUM tiles (one elementwise op per layer), biases
    are preloaded into PSUM off the critical path, and the attention phase is
    interleaved into the scan to fill engine gaps and keep the PE clock warm.
"""

import sys

import numpy as np

_TRN = "/opt/trn_rl_repo"
if _TRN not in sys.path:
    sys.path.insert(0, _TRN)

B, T, D, H, L = 64, 512, 128, 128, 256
NCORES = 8
BPC = B // NCORES          # batches per core
NSTEPS = T - 1             # 511 scan steps
DT = np.float32(1.0 / NSTEPS)
SQDT = np.float32(np.sqrt(DT))
SCAN_DT = "f16"            # "f16" or "f8" — low-precision dtype for the SDE scan

_prog_cache = {}


def _f(a):
    return np.ascontiguousarray(np.asarray(a, dtype=np.float64))


def _rep8(col):
    return np.tile(np.asarray(col, np.float32).reshape(-1, 1), (1, BPC))


def fold_params(params):
    """Host-side weight folding in float64, cast to on-chip dtypes."""
    w = {}
    s = 1.0 / np.sqrt(D)
    attn = params["attn"]
    w["wq"] = (_f(attn["q"]["w"]) * s).astype(np.float32)
    w["bq"] = (_f(attn["q"]["b"]) * s).astype(np.float32).reshape(D, 1)
    w["wk"] = _f(attn["k"]["w"]).astype(np.float32)
    w["bk"] = _f(attn["k"]["b"]).astype(np.float32).reshape(D, 1)
    w["wv"] = _f(attn["v"]["w"]).astype(np.float32)
    w["bv"] = _f(attn["v"]["b"]).astype(np.float32).reshape(D, 1)

    if SCAN_DT == "f8":
        import ml_dtypes
        np16 = ml_dtypes.float8_e4m3
    else:
        np16 = np.float16
    bias = {}
    for tag, net in (("d", params["drift"]), ("f", params["diff"])):
        win, bin_ = _f(net["in"]["w"]), _f(net["in"]["b"])
        w0, b0 = _f(net["fc"][0]["w"]), _f(net["fc"][0]["b"])
        w1, b1 = _f(net["fc"][1]["w"]), _f(net["fc"][1]["b"])
        wo, bo = _f(net["out"]["w"]), _f(net["out"]["b"])
        w[f"w{tag}1"] = (win @ w0).astype(np16)                        # 128x256
        bias[f"{tag}1"] = (bin_ @ w0 + b0).astype(np.float32)
        w[f"w{tag}2"] = w1.astype(np16)                                # 256x256
        bias[f"{tag}2"] = b1.astype(np.float32)
        w[f"w{tag}3"] = wo.astype(np16)                                # 256x128
        bias[f"{tag}3"] = bo.astype(np.float32)
    # PSUM bias preload tiles: joint layout [d-m0 | d-m1 | f-m0 | f-m1] x 8 cols
    w["bw1"] = np.concatenate(
        [_rep8(bias["d1"][:128]), _rep8(bias["d1"][128:]),
         _rep8(bias["f1"][:128]), _rep8(bias["f1"][128:])], axis=1)
    w["bw2"] = np.concatenate(
        [_rep8(bias["d2"][:128]), _rep8(bias["d2"][128:]),
         _rep8(bias["f2"][:128]), _rep8(bias["f2"][128:])], axis=1)
    w["bw3"] = np.concatenate([_rep8(bias["f3"]), _rep8(bias["d3"])], axis=1)

    wfi, bfi = _f(params["fc_in"]["w"]), _f(params["fc_in"]["b"])
    wb0, bb0 = _f(params["fc_block"][0]["w"]), _f(params["fc_block"][0]["b"])
    wb1, bb1 = _f(params["fc_block"][1]["w"]), _f(params["fc_block"][1]["b"])
    wo, bo = _f(params["fc_out"]["w"]), _f(params["fc_out"]["b"])
    w["wu1"] = (wfi @ wb0).astype(np16)                                # 256x256
    w["bu1"] = (bfi @ wb0 + bb0).astype(np.float32).reshape(L, 1)
    w["wu2"] = wb1.astype(np16)                                        # 256x256
    w["bu2"] = bb1.astype(np.float32).reshape(L, 1)
    w["wu3"] = wo.astype(np16)                                         # 256x128
    w["bu3"] = bo.astype(np.float32).reshape(D, 1)
    return w


def build_program(n_steps=NSTEPS, do_scan=True, do_attn=True, debug=False,
                  scan_dt=None, attn_stride=56, FUSE_ILV=False):
    scan_dt = scan_dt or SCAN_DT
    import concourse.bacc as bacc
    import concourse.tile as tile
    from concourse import masks, mybir
    from contextlib import ExitStack

    f32 = mybir.dt.float32
    f16 = {"f16": mybir.dt.float16, "f8": mybir.dt.float8e4}[scan_dt]
    AF = mybir.ActivationFunctionType
    ALU = mybir.AluOpType

    Tloc = n_steps + 1

    nc = bacc.Bacc("TRN2", target_bir_lowering=False, debug=False)

    # ---- DRAM I/O ----
    x_d = nc.dram_tensor("x", (BPC, T, D), f32, kind="ExternalInput")
    nz_d = nc.dram_tensor("noise", (NSTEPS, BPC, H), f32, kind="ExternalInput")
    dram = {}
    for name, shape, dt in [
        ("wq", (D, D), f32), ("bq", (D, 1), f32),
        ("wk", (D, D), f32), ("bk", (D, 1), f32),
        ("wv", (D, D), f32), ("bv", (D, 1), f32),
        ("wd1", (H, L), f16), ("wd2", (L, L), f16), ("wd3", (L, H), f16),
        ("wf1", (H, L), f16), ("wf2", (L, L), f16), ("wf3", (L, H), f16),
        ("bw1", (128, 4 * BPC), f32), ("bw2", (128, 4 * BPC), f32),
        ("bw3", (128, 2 * BPC), f32),
        ("wu1", (L, L), f16), ("bu1", (L, 1), f32),
        ("wu2", (L, L), f16), ("bu2", (L, 1), f32),
        ("wu3", (L, D), f16), ("bu3", (D, 1), f32),
    ]:
        dram[name] = nc.dram_tensor(name, shape, dt, kind="ExternalInput")
    out_d = nc.dram_tensor("out", (BPC, T, D), f32, kind="ExternalOutput")
    if debug:
        hidT_d = nc.dram_tensor("dbg_hidT", (H, BPC, Tloc), f32, kind="ExternalOutput")
        ctxT_d = nc.dram_tensor("dbg_ctxT", (BPC, D, T), f32, kind="ExternalOutput")

    with tile.TileContext(nc) as tc, ExitStack() as octx:
        const = octx.enter_context(tc.tile_pool(name="const", bufs=1))

        ident = const.tile([128, 128], f32)
        masks.make_identity(nc, ident[:])

        sb = {}
        for name in ("wq", "wk", "wv"):
            sb[name] = const.tile([D, D], f32, name=name)
            nc.sync.dma_start(out=sb[name], in_=dram[name].ap())
        for name in ("bq", "bk", "bv", "bu3"):
            sb[name] = const.tile([128, 1], f32, name=name)
            nc.sync.dma_start(out=sb[name], in_=dram[name].ap())
        for name in ("bw1", "bw2", "bw3"):
            shape = [128, 4 * BPC] if name != "bw3" else [128, 2 * BPC]
            sb[name] = const.tile(shape, f32, name=name)
            nc.sync.dma_start(out=sb[name], in_=dram[name].ap())
        # SDE f16 weights; K>128 stored [128, kc, M]
        for tag in ("d", "f"):
            sb[f"w{tag}1"] = const.tile([H, L], f16, name=f"w{tag}1")
            nc.sync.dma_start(out=sb[f"w{tag}1"], in_=dram[f"w{tag}1"].ap())
            sb[f"w{tag}2"] = const.tile([128, 2, L], f16, name=f"w{tag}2")
            nc.sync.dma_start(
                out=sb[f"w{tag}2"],
                in_=dram[f"w{tag}2"].ap().rearrange("(c p) m -> p c m", p=128))
            sb[f"w{tag}3"] = const.tile([128, 2, H], f16, name=f"w{tag}3")
            nc.sync.dma_start(
                out=sb[f"w{tag}3"],
                in_=dram[f"w{tag}3"].ap().rearrange("(c p) m -> p c m", p=128))
        # fusion f32 weights
        for name in ("wu1", "wu2"):
            sb[name] = const.tile([128, 2, L], f16, name=name)
            nc.sync.dma_start(
                out=sb[name],
                in_=dram[name].ap().rearrange("(c p) m -> p c m", p=128))
        sb["wu3"] = const.tile([128, 2, D], f16, name="wu3")
        nc.sync.dma_start(
            out=sb["wu3"], in_=dram["wu3"].ap().rearrange("(c p) m -> p c m", p=128))
        for name in ("bu1", "bu2"):
            sb[name] = const.tile([128, 2], f32, name=name)
            nc.sync.dma_start(
                out=sb[name],
                in_=dram[name].ap().rearrange("(c p) one -> p (c one)", p=128))

        # persistent state
        hidden = const.tile([H, BPC, Tloc], f32)   # hidden^T, batch-major cols
        hidden16 = const.tile([H, BPC, Tloc], f16)  # fp16 copy for the fusion
        noiseT = const.tile([H, n_steps * BPC], f32)  # sqrt(dt)*dw, feature-major
        ctxT_l = [const.tile([D, T], f16, name=f"ctxT{b}") for b in range(BPC)]

        # pools shared by scan + attention so they can interleave
        scn = octx.enter_context(tc.tile_pool(name="scn", bufs=2))
        scnp = octx.enter_context(tc.tile_pool(name="scnp", bufs=1, space="PSUM"))
        atp = octx.enter_context(tc.tile_pool(name="atp", bufs=2))
        apb = octx.enter_context(tc.tile_pool(name="apb", bufs=1, space="PSUM"))
        aps = octx.enter_context(tc.tile_pool(name="aps", bufs=1, space="PSUM"))
        ntp = octx.enter_context(tc.tile_pool(name="ntp", bufs=3))

        def emit_noise_block(t0):
            tcnt = min(16, n_steps - t0)
            rows = tcnt * BPC
            nz = ntp.tile([128, H], f32, tag="nz")
            nc.sync.dma_start(
                out=nz[:rows, :],
                in_=nz_d.ap()[t0:t0 + tcnt].rearrange("t b h -> (t b) h"))
            ps = aps.tile([128, 128], f32, tag="tps")
            nc.tensor.transpose(ps[:, :rows], nz[:rows, :], ident[:rows, :rows])
            nc.scalar.activation(
                out=noiseT[:, t0 * BPC: t0 * BPC + rows], in_=ps[:, :rows],
                func=AF.Copy, scale=float(SQDT))

        for t0 in range(0, min(48, n_steps), 16):
            emit_noise_block(t0)

        fusion_done = set()

        def emit_attn(b):
            """Attention part A for batch b (independent of the scan)."""
            xt = atp.tile([128, 4, D], f32, tag="xt")
            nc.sync.dma_start(
                out=xt, in_=x_d.ap()[b].rearrange("(c p) d -> p c d", p=128))
            xT = atp.tile([D, T], f32, tag="xT")
            for c in range(4):
                ps = aps.tile([128, 128], f32, tag="tps")
                nc.tensor.transpose(ps, xt[:, c, :], ident)
                nc.scalar.copy(xT[:, c * 128:(c + 1) * 128], ps)
            qT = atp.tile([D, T], f32, tag="qT")
            kT = atp.tile([D, T], f32, tag="kT")
            for wname, bname, dst in (("wq", "bq", qT), ("wk", "bk", kT)):
                ps = apb.tile([D, T], f32, tag="bps")
                nc.tensor.matmul(ps, lhsT=sb[wname], rhs=xT, start=True, stop=True)
                nc.scalar.activation(out=dst, in_=ps, func=AF.Identity,
                                     bias=sb[bname])
            v = atp.tile([128, 4, D], f32, tag="v")
            for c in range(4):
                ps = aps.tile([128, 128], f32, tag="tps")
                nc.tensor.matmul(ps, lhsT=xT[:, c * 128:(c + 1) * 128],
                                 rhs=sb["wv"], start=True, stop=True)
                nc.scalar.copy(v[:, c, :], ps)
            # scores (t-major) -> exp (+fused row-sum) -> normalize
            P = atp.tile([128, 4, T], f32, tag="P")
            sums = atp.tile([128, 4], f32, tag="sums")
            for c in range(4):
                ps = apb.tile([128, T], f32, tag="bps")
                nc.tensor.matmul(ps, lhsT=qT[:, c * 128:(c + 1) * 128],
                                 rhs=kT, start=True, stop=True)
                nc.scalar.activation(out=P[:, c, :], in_=ps, func=AF.Exp,
                                     accum_out=sums[:, c:c + 1])
            rec = atp.tile([128, 4], f32, tag="rec")
            nc.vector.reciprocal(rec, sums)
            for c in range(4):
                nc.vector.tensor_scalar_mul(P[:, c, :], P[:, c, :],
                                            rec[:, c:c + 1])
            # transpose P -> PT (s-major)
            PT = atp.tile([128, 4, T], f32, tag="PT")
            for tc4 in range(4):
                for sc in range(4):
                    ps = aps.tile([128, 128], f32, tag="tps")
                    nc.tensor.transpose(
                        ps, P[:, tc4, sc * 128:(sc + 1) * 128], ident)
                    nc.scalar.copy(PT[:, sc, tc4 * 128:(tc4 + 1) * 128], ps)
            # context^T = sum_s v[s,:]^T P^T[s,:] (+ bv)
            psc = apb.tile([D, T], f32, tag="bps")
            for sc in range(4):
                nc.tensor.matmul(psc, lhsT=v[:, sc, :], rhs=PT[:, sc, :],
                                 start=(sc == 0), stop=(sc == 3))
            nc.scalar.activation(out=ctxT_l[b], in_=psc, func=AF.Identity,
                                 bias=sb["bv"])
            if debug:
                nc.sync.dma_start(out=ctxT_d.ap()[b], in_=ctxT_l[b])

        TH = T

        def emit_fusion(b, half):
            tlo = half * TH
            hu1 = atp.tile([128, 2, TH], f16, tag="hu1")
            for mi in range(2):
                ps = scnp.tile([128, TH], f32, tag="pj1", bufs=2)
                nc.tensor.matmul(
                    ps, lhsT=sb["wu1"][:, 0, mi * 128:(mi + 1) * 128],
                    rhs=ctxT_l[b][:, tlo:tlo + TH], start=True, stop=False)
                nc.tensor.matmul(
                    ps, lhsT=sb["wu1"][:, 1, mi * 128:(mi + 1) * 128],
                    rhs=hidden16[:, b, tlo:tlo + TH], start=False, stop=True)
                nc.vector.tensor_scalar(
                    out=hu1[:, mi, :], in0=ps, scalar1=sb["bu1"][:, mi:mi + 1],
                    scalar2=0.0, op0=ALU.add, op1=ALU.max)
            hu2 = atp.tile([128, 2, TH], f16, tag="hu2")
            for mi in range(2):
                ps = scnp.tile([128, TH], f32, tag="pj2", bufs=2)
                nc.tensor.matmul(
                    ps, lhsT=sb["wu2"][:, 0, mi * 128:(mi + 1) * 128],
                    rhs=hu1[:, 0, :], start=True, stop=False)
                nc.tensor.matmul(
                    ps, lhsT=sb["wu2"][:, 1, mi * 128:(mi + 1) * 128],
                    rhs=hu1[:, 1, :], start=False, stop=True)
                nc.vector.tensor_scalar(
                    out=hu2[:, mi, :], in0=ps, scalar1=sb["bu2"][:, mi:mi + 1],
                    scalar2=0.0, op0=ALU.add, op1=ALU.max)
            pso = scnp.tile([D, TH], f32, tag="pj3", bufs=2)
            nc.tensor.matmul(pso, lhsT=sb["wu3"][:, 0, :], rhs=hu2[:, 0, :],
                             start=True, stop=False)
            nc.tensor.matmul(pso, lhsT=sb["wu3"][:, 1, :], rhs=hu2[:, 1, :],
                             start=False, stop=True)
            outT = atp.tile([D, TH], f32, tag="outT")
            nc.scalar.activation(out=outT, in_=pso, func=AF.Identity,
                                 bias=sb["bu3"])
            ot = atp.tile([128, TH // 128, D], f32, tag="ot")
            for c in range(TH // 128):
                ps = aps.tile([128, 128], f32, tag="tps")
                nc.tensor.transpose(ps, outT[:, c * 128:(c + 1) * 128], ident)
                nc.scalar.copy(ot[:, c, :], ps)
            nc.sync.dma_start(
                out=out_d.ap()[b][tlo:tlo + TH].rearrange("(c p) d -> p c d", p=128),
                in_=ot)

        # ---- phase 2: SDE scan (with attention part A interleaved) ----
        attn_queue = list(range(BPC)) if (do_attn and n_steps == T - 1) else []
        if do_scan:
            nc.vector.memset(hidden[:, :, 0], 0.0)
            y16_0 = scn.tile([H, BPC], f16, tag="y16")
            nc.vector.memset(y16_0, 0.0)
            y16 = y16_0
            W1 = {t: sb[f"w{t}1"] for t in ("d", "f")}
            W2 = {t: sb[f"w{t}2"] for t in ("d", "f")}
            W3 = {t: sb[f"w{t}3"] for t in ("d", "f")}
            # fc1 is linear in the update, so p1(t+1) accumulates
            # bias + W1.y(t) (early) + dt.W1.sigd + W1.(sigf*dw) (late);
            # only the sigf*dw term is on the step's critical path.
            def fc1_accum(dst, rhs, suffix="", stop=False):
                for ni, tag in enumerate(("d", "f")):
                    for mi in range(2):
                        nc.tensor.matmul(
                            dst[:, (ni * 2 + mi) * BPC:(ni * 2 + mi + 1) * BPC],
                            lhsT=sb[f"w{tag}1{suffix}"][:, mi * 128:(mi + 1) * 128],
                            rhs=rhs, start=False, stop=stop,
                            skip_group_check=True)

            p1 = scnp.tile([128, 4 * BPC], f32, tag="pj1", bufs=2)
            nc.vector.tensor_copy(p1, sb["bw1"])
            fc1_accum(p1, y16, stop=True)
            for t in range(n_steps):
                if t % 16 == 0 and t + 48 < n_steps:
                    emit_noise_block(t + 48)
                if attn_queue and t == 24 + (BPC - len(attn_queue)) * attn_stride:
                    emit_attn(attn_queue.pop(0))
                if (FUSE_ILV and do_attn and n_steps == T - 1 and t >= 328
                        and t % 16 == 8 and len(fusion_done) < BPC):
                    b = len(fusion_done)
                    emit_fusion(b, 0)
                    fusion_done.add((b, 0))
                # start next step's p1 with the terms known early
                p1n = scnp.tile([128, 4 * BPC], f32, tag="pj1", bufs=2)
                nc.vector.tensor_copy(p1n, sb["bw1"])
                fc1_accum(p1n, y16)
                h1 = scn.tile([128, 4 * BPC], f16, tag="h1")
                nc.vector.tensor_scalar(out=h1, in0=p1, scalar1=0.0, scalar2=None,
                                        op0=ALU.max)
                p2 = scnp.tile([128, 4 * BPC], f32, tag="pj2", bufs=2)
                nc.scalar.copy(p2, sb["bw2"])
                for ni, tag in enumerate(("d", "f")):
                    for mi in range(2):
                        reg = p2[:, (ni * 2 + mi) * BPC:(ni * 2 + mi + 1) * BPC]
                        nc.tensor.matmul(
                            reg, lhsT=W2[tag][:, 0, mi * 128:(mi + 1) * 128],
                            rhs=h1[:, ni * 2 * BPC:(ni * 2 + 1) * BPC],
                            start=False, stop=False, skip_group_check=True)
                        nc.tensor.matmul(
                            reg, lhsT=W2[tag][:, 1, mi * 128:(mi + 1) * 128],
                            rhs=h1[:, (ni * 2 + 1) * BPC:(ni * 2 + 2) * BPC],
                            start=False, stop=True, skip_group_check=True)
                h2 = scn.tile([128, 4 * BPC], f16, tag="h2")
                nc.vector.tensor_scalar(out=h2, in0=p2, scalar1=0.0, scalar2=None,
                                        op0=ALU.max)
                # p3 joint [f | d]: diff first so the noise-multiply
                # (which gates y16) starts as early as possible
                p3 = scnp.tile([H, 2 * BPC], f32, tag="pj3", bufs=2)
                nc.scalar.copy(p3, sb["bw3"])
                hslc = {"d": (h2[:, 0:BPC], h2[:, BPC:2 * BPC]),
                        "f": (h2[:, 2 * BPC:3 * BPC], h2[:, 3 * BPC:4 * BPC])}
                for ni, tag in enumerate(("f", "d")):
                    reg = p3[:, ni * BPC:(ni + 1) * BPC]
                    nc.tensor.matmul(
                        reg, lhsT=W3[tag][:, 0, :], rhs=hslc[tag][0],
                        start=False, stop=False, skip_group_check=True)
                    nc.tensor.matmul(
                        reg, lhsT=W3[tag][:, 1, :], rhs=hslc[tag][1],
                        start=False, stop=True, skip_group_check=True)
                sg = scn.tile([H, 2 * BPC], f32, tag="sg")
                nc.scalar.activation(out=sg, in_=p3, func=AF.Sigmoid)
                # critical path: delta = dt*sig_d + sig_f*dw as one fp16 rhs
                ftmp = scn.tile([H, BPC], f32, tag="ftmp")
                nc.vector.tensor_mul(
                    ftmp, sg[:, 0:BPC], noiseT[:, t * BPC:(t + 1) * BPC])
                d16 = scn.tile([H, BPC], f16, tag="d16")
                nc.vector.scalar_tensor_tensor(
                    out=d16, in0=sg[:, BPC:2 * BPC], scalar=float(DT),
                    in1=ftmp, op0=ALU.mult, op1=ALU.add)
                fc1_accum(p1n, d16, stop=True)
                # off critical path: exact fp32 state update
                ytmp = scn.tile([H, BPC], f32, tag="ytmp")
                nc.vector.scalar_tensor_tensor(
                    out=ytmp, in0=sg[:, BPC:2 * BPC], scalar=float(DT),
                    in1=hidden[:, :, t], op0=ALU.mult, op1=ALU.add)
                nc.gpsimd.tensor_add(hidden[:, :, t + 1], ytmp, ftmp)
                y16 = scn.tile([H, BPC], f16, tag="y16")
                nc.vector.tensor_add(y16, ytmp, ftmp)
                p1 = p1n
        else:
            nc.vector.memset(hidden[:, :, :], 0.0)
        for b in attn_queue:
            emit_attn(b)
        if debug:
            nc.sync.dma_start(out=hidT_d.ap(), in_=hidden)

        # fp16 copy of hidden for the fusion matmuls
        if do_attn:
            half_cols = BPC * Tloc // 2
            hv32 = hidden.rearrange("p b t -> p (b t)")
            hv16 = hidden16.rearrange("p b t -> p (b t)")
            nc.vector.tensor_copy(hv16[:, 0:half_cols], hv32[:, 0:half_cols])
            nc.vector.tensor_copy(hv16[:, half_cols:], hv32[:, half_cols:])

        # ---- phase 3 epilogue: remaining fusion halves ----
        if do_attn:
            assert Tloc == T
            for b in range(BPC):
                emit_fusion(b, 0)
    nc.compile()
    return nc


def _get_program():
    key = ("full", SCAN_DT)
    if key not in _prog_cache:
        _prog_cache[key] = build_program()
    return _prog_cache[key]


def kernel(x, noise, params):
    from concourse import bass_utils

    x = np.ascontiguousarray(np.asarray(x, dtype=np.float32))
    noise = np.ascontiguousarray(np.asarray(noise, dtype=np.float32))
    w = fold_params(params)

    nc = _get_program()
    in_maps = []
    for c in range(NCORES):
        m = dict(w)
        m["x"] = np.ascontiguousarray(x[c * BPC:(c + 1) * BPC])
        m["noise"] = np.ascontiguousarray(noise[:, c * BPC:(c + 1) * BPC])
        in_maps.append(m)
    res = bass_utils.run_bass_kernel_spmd(nc, in_maps, core_ids=list(range(NCORES)))
    return np.concatenate([res.results[c]["out"] for c in range(NCORES)], axis=0)


if __name__ == "__main__":
    import pickle, time
    x = np.load("/tmp/x.npy")
    noise = np.load("/tmp/noise.npy")
    with open("/tmp/params.pkl", "rb") as f:
        params = pickle.load(f)
    t0 = time.time()
    out = kernel(x, noise, params)
    print("kernel() wall:", time.time() - t0)
    ref = np.load("/tmp/ref_np.npy")
    err = np.abs(out - ref)
    print("absmax:", err.max(), "rel:", err.max() / np.abs(ref).max())
